# revision 51
# baseline (speedup 1.0000x reference)
"""FCOS head (nms_detection) Trainium2 Bass kernel.

Strategy: data-parallel over batch across 8 NeuronCores (1 image/core,
weights replicated).  Per core, each 3x3 SAME conv is computed as 18
accumulating float32r matmuls (9 taps x 2 input-channel halves) per
<=512-pixel output tile, reading from a zero-padded SBUF activation
image.  GroupNorm statistics come from bn_stats/bn_aggr per channel,
then two tiny exact-fp32 matmuls do the cross-partition group reduce
and broadcast; normalize+ReLU is a single fused scalar-engine
activation (Relu(A*x+B)) per channel-half.  Level 0 (100x128) streams
conv outputs through a DRAM scratch buffer (its x and y don't both fit
in SBUF); levels 1-4 stay SBUF-resident.  Head convs (cls / box+ctr)
use the same tap-matmul scheme with M=16 / M=5.
"""

import math

import numpy as np

# ---------------------------------------------------------------- constants
IN_CH = 256
NUM_CLASSES = 16
NUM_CONVS = 4
STRIDES = (8, 16, 32, 64, 128)
GN_EPS = 1e-5
NCORES = 8

# (H, W) per level
LEVELS = [(100, 128), (50, 64), (25, 32), (13, 16), (7, 8)]
# output-row chunk per level (rows*W <= 512).  Chunk sizes may be unequal;
# gn stats aggregate per size-group and combine with host-known weights.
ROWS = [4, 8, 16, 13, 7]
OFFS = [0, 12800, 16000, 16800, 17008]
TOTAL_LOCS = 17064

F0_REFILL_ROWS = 10  # rows per refill chunk for level 0


def _np(x):
    return np.ascontiguousarray(np.asarray(x), dtype=np.float32)


def _tiles_for(level):
    H, _ = LEVELS[level]
    R = ROWS[level]
    out = []
    r = 0
    while r < H:
        out.append((r, min(R, H - r)))
        r += R
    return out


def _prep_tower_w(W):
    # W [O=256, I=256, 3, 3] -> [p=128, kh=2, tap=9, mh=2, m=128] flattened
    W = _np(W).reshape(2, 128, 2, 128, 3, 3)  # [mh, m, kh, p, dy, dx]
    wt = np.transpose(W, (3, 2, 4, 5, 0, 1))  # [p, kh, dy, dx, mh, m]
    return np.ascontiguousarray(wt.reshape(128, 2 * 9 * 2 * 128))


def _prep_head_w(Ws):
    # list of [o_i, 256, 3, 3] stacked on o -> [p, kh, tap, m_total]
    W = np.concatenate([_np(w) for w in Ws], axis=0)  # [M, 256, 3, 3]
    M = W.shape[0]
    W = W.reshape(M, 2, 128, 3, 3)  # [m, kh, p, dy, dx]
    wt = np.transpose(W, (2, 1, 3, 4, 0))  # [p, kh, dy, dx, m]
    return np.ascontiguousarray(wt.reshape(128, 2 * 9 * M)), M


def _locations():
    locs = []
    for l, (h, w) in enumerate(LEVELS):
        s = STRIDES[l]
        sx = np.arange(0, w * s, s, dtype=np.float32)
        sy = np.arange(0, h * s, s, dtype=np.float32)
        yy, xx = np.meshgrid(sy, sx, indexing="ij")
        locs.append(np.stack([xx.reshape(-1), yy.reshape(-1)], axis=1) + s // 2)
    return np.concatenate(locs, axis=0).astype(np.float32)


# ---------------------------------------------------------------- program
def build_program(scales):
    import concourse.bacc as bacc
    import concourse.mybir as mybir
    import concourse.tile as tile

    f32 = mybir.dt.float32
    f32r = mybir.dt.float32r
    AF = mybir.ActivationFunctionType

    nc = bacc.Bacc(trn_type="TRN2", num_swdge_queues=4)

    feats = [
        nc.declare_dram_parameter(f"f{l}", [IN_CH, H, W], f32, isOutput=False)
        for l, (H, W) in enumerate(LEVELS)
    ]
    wts = {}
    pks = {}
    for t in ("c", "b"):
        for L in range(NUM_CONVS):
            wts[(t, L)] = nc.declare_dram_parameter(
                f"w{t}{L}", [128, 4608], f32, isOutput=False
            )
            pks[(t, L)] = nc.declare_dram_parameter(
                f"p{t}{L}", [128, 6], f32, isOutput=False
            )
    wch = nc.declare_dram_parameter("wch", [128, 288], f32, isOutput=False)
    wbh = nc.declare_dram_parameter("wbh", [128, 90], f32, isOutput=False)
    hbc = nc.declare_dram_parameter("hbc", [16, 1], f32, isOutput=False)
    hbb = nc.declare_dram_parameter("hbb", [4, 5], f32, isOutput=False)
    ind16_d = nc.declare_dram_parameter("ind16", [128, 16], f32, isOutput=False)
    bc16_d = nc.declare_dram_parameter("bc16", [16, 128], f32, isOutput=False)
    out = nc.declare_dram_parameter("out", [21, TOTAL_LOCS], f32, isOutput=True)

    with tile.TileContext(nc) as tc:
        # ---- constants that live for the whole kernel
        with tc.tile_pool(name="konst", bufs=1) as kp:
            ind16 = kp.tile([128, 16], f32)
            nc.sync.dma_start(out=ind16, in_=ind16_d[:, :])
            bc16 = kp.tile([16, 128], f32)
            nc.sync.dma_start(out=bc16, in_=bc16_d[:, :])
            hbc_t = kp.tile([16, 1], f32)
            nc.sync.dma_start(out=hbc_t, in_=hbc[:, :])
            hbb_t = kp.tile([4, 5], f32)
            nc.sync.dma_start(out=hbb_t, in_=hbb[:, :])
            epst = kp.tile([128, 1], f32)
            nc.vector.memset(epst, GN_EPS)

            def conv_tile(ps, wt_t, xp, level, kind, r0, R, width=None):
                """Accumulate 3x3 conv for output rows [r0, r0+R) into psum.

                kind: ("tower", mh) or ("head", M) selecting lhsT slice.
                wt_t viewed as [128, 2, 9, X]."""
                W = width if width is not None else LEVELS[level][1]
                first = True
                for kh in range(2):
                    for tap in range(9):
                        dy, dx = tap // 3, tap % 3
                        rhs = xp[:, kh, r0 + dy : r0 + dy + R, dx : dx + W]
                        if kind[0] == "tower":
                            lhsT = wt_t[:, kh, tap, kind[1], :]
                        else:
                            lhsT = wt_t[:, kh, tap, : kind[1]]
                        nc.tensor.matmul(
                            ps,
                            lhsT,
                            rhs,
                            start=first,
                            stop=(kh == 1 and tap == 8),
                        )
                        first = False

            def gn_finalize(spool, pspool, stats_t, pk_t, tile_list, W, tagsfx=""):
                """Per-channel chunk stats -> per-channel affine (A,B) [128,4].

                Chunks may have unequal pixel counts: bn_aggr is unweighted, so
                aggregate per size-group and combine with host-known weights."""
                Ns = [R * W for (_, R) in tile_list]
                groups = []  # (ti_start, ti_end, chunk_px)
                for ti, n in enumerate(Ns):
                    if groups and groups[-1][2] == n:
                        groups[-1][1] = ti + 1
                    else:
                        groups.append([ti, ti + 1, n])
                total = float(sum(Ns))
                # me cols: mean0 e2_0 mean1 e2_1 (bias-free mean and E[x^2])
                me = spool.tile([128, 4], f32, tag="me" + tagsfx)
                sq = spool.tile([128, 1], f32, tag="sq" + tagsfx)
                mv = spool.tile([128, 2], f32, tag="mv" + tagsfx)
                acc = spool.tile([128, 2], f32, tag="macc" + tagsfx)
                for h in range(2):
                    if len(groups) == 1:
                        nc.vector.bn_aggr(out=mv, in_=stats_t[:, h])
                        nc.vector.tensor_copy(me[:, 2 * h : 2 * h + 1], mv[:, 0:1])
                        nc.vector.tensor_mul(sq, mv[:, 0:1], mv[:, 0:1])
                        nc.vector.tensor_add(
                            me[:, 2 * h + 1 : 2 * h + 2], mv[:, 1:2], sq
                        )
                    else:
                        first = True
                        for (s, e, n) in groups:
                            w_g = (e - s) * n / total
                            nc.vector.bn_aggr(out=mv, in_=stats_t[:, h, s:e])
                            # e2_g = var + mean^2 ; acc += w_g * [mean, e2_g]
                            nc.vector.tensor_mul(sq, mv[:, 0:1], mv[:, 0:1])
                            nc.vector.tensor_add(mv[:, 1:2], mv[:, 1:2], sq)
                            nc.vector.tensor_scalar_mul(mv, mv, w_g)
                            if first:
                                nc.vector.tensor_copy(acc, mv)
                                first = False
                            else:
                                nc.vector.tensor_add(acc, acc, mv)
                        nc.vector.tensor_copy(me[:, 2 * h : 2 * h + 1], acc[:, 0:1])
                        nc.vector.tensor_copy(
                            me[:, 2 * h + 1 : 2 * h + 2], acc[:, 1:2]
                        )
                # mbuf cols: m0 s0 m1 s1 ; m = mean + b, s = e2 + b*(mean + m)
                mbuf = spool.tile([128, 4], f32, tag="mbuf" + tagsfx)
                for h in range(2):
                    nc.vector.tensor_add(
                        mbuf[:, 2 * h : 2 * h + 1], me[:, 2 * h : 2 * h + 1],
                        pk_t[:, h, 2:3],
                    )
                    nc.vector.tensor_add(
                        sq, me[:, 2 * h : 2 * h + 1], mbuf[:, 2 * h : 2 * h + 1]
                    )
                    nc.vector.tensor_mul(sq, sq, pk_t[:, h, 2:3])
                    nc.vector.tensor_add(
                        mbuf[:, 2 * h + 1 : 2 * h + 2],
                        me[:, 2 * h + 1 : 2 * h + 2], sq,
                    )
                gps = pspool.tile([128, 4], f32, tag="pstat")
                nc.tensor.matmul(gps[:16, :], ind16, mbuf, start=True, stop=True)
                gsb = spool.tile([16, 4], f32, tag="gsb" + tagsfx)
                nc.scalar.activation(gsb, gps[:16, :], AF.Copy)
                bps = pspool.tile([128, 4], f32, tag="pstat")
                nc.tensor.matmul(bps, bc16, gsb, start=True, stop=True)
                bsb = spool.tile([128, 4], f32, tag="bsb" + tagsfx)
                nc.scalar.activation(bsb, bps, AF.Copy)
                # var = E2 - mu^2 ; rstd = 1/sqrt(var+eps)
                ab = spool.tile([128, 4], f32, tag="ab" + tagsfx)  # A0 A1 B0 B1
                var = spool.tile([128, 2], f32, tag="var" + tagsfx)
                for h in range(2):
                    nc.vector.tensor_mul(sq, bsb[:, 2 * h : 2 * h + 1], bsb[:, 2 * h : 2 * h + 1])
                    nc.vector.tensor_sub(var[:, h : h + 1], bsb[:, 2 * h + 1 : 2 * h + 2], sq)
                    nc.scalar.activation(
                        var[:, h : h + 1], var[:, h : h + 1], AF.Sqrt, bias=epst[:, 0:1]
                    )
                    nc.vector.reciprocal(var[:, h : h + 1], var[:, h : h + 1])
                    nc.vector.tensor_mul(
                        ab[:, h : h + 1], var[:, h : h + 1], pk_t[:, h, 0:1]
                    )
                    # B = beta + A*(bias - mu)
                    nc.vector.tensor_sub(sq, pk_t[:, h, 2:3], bsb[:, 2 * h : 2 * h + 1])
                    nc.vector.tensor_mul(sq, ab[:, h : h + 1], sq)
                    nc.vector.tensor_add(ab[:, 2 + h : 3 + h], pk_t[:, h, 1:2], sq)
                return ab

            def memset_borders(xp, level):
                H, W = LEVELS[level]
                for h in range(2):
                    nc.vector.memset(xp[:, h, 0:1, :].bitcast(f32), 0.0)
                    nc.vector.memset(xp[:, h, H + 1 : H + 2, :].bitcast(f32), 0.0)
                    nc.vector.memset(xp[:, h, 1 : H + 1, 0:1].bitcast(f32), 0.0)
                    nc.vector.memset(
                        xp[:, h, 1 : H + 1, W + 1 : W + 2].bitcast(f32), 0.0
                    )

            def head_convs(xp, level, pspool, hpool, wch_t, wbh_t, interleave=None):
                """cls head if wch_t else box+ctr head, from padded tower out.

                interleave: optional {tile_idx: callable} of filler work to
                emit between head tiles (keeps engine FIFOs pipelined)."""
                H, W = LEVELS[level]
                off = OFFS[level]
                for ti, (r0, R) in enumerate(_tiles_for(level)):
                    N = R * W
                    cols = slice(off + r0 * W, off + r0 * W + N)
                    if wch_t is not None:
                        ps = pspool.tile([128, 512], f32, tag=f"ps{ti % 2}")
                        conv_tile(
                            ps[:16, :N].rearrange("p (r w) -> p r w", r=R),
                            wch_t, xp, level, ("head", 16), r0, R,
                        )
                        hs = hpool.tile([16, 512], f32, tag="hscls")
                        nc.vector.tensor_scalar_add(hs[:, :N], ps[:16, :N], hbc_t[:, 0:1])
                        nc.sync.dma_start(out=out[0:16, cols], in_=hs[:, :N])
                    else:
                        s = float(scales[level]) * float(STRIDES[level])
                        ps = pspool.tile([128, 512], f32, tag=f"ps{ti % 2}")
                        conv_tile(
                            ps[:5, :N].rearrange("p (r w) -> p r w", r=R),
                            wbh_t, xp, level, ("head", 5), r0, R,
                        )
                        hs = hpool.tile([5, 512], f32, tag="hsbc")
                        hr = hpool.tile([5, 512], f32, tag="hsraw")
                        nc.vector.tensor_copy(hr[:, :N], ps[:5, :N])
                        # box rows: relu(s*t*x + b*s*t) = s*t*relu(x + b), s,t>0
                        nc.vector.tensor_scalar(
                            hs[:4, :N], ps[:4, :N],
                            s, hbb_t[:, level : level + 1],
                            op0=mybir.AluOpType.mult, op1=mybir.AluOpType.add,
                        )
                        nc.vector.tensor_scalar_max(hs[:4, :N], hs[:4, :N], 0.0)
                        nc.sync.dma_start(out=out[16:20, cols], in_=hs[:4, :N])
                        nc.sync.dma_start(out=out[20:21, cols], in_=hr[4:5, :N])
                    if interleave and ti in interleave:
                        interleave[ti]()

            # ---- combined level 3+4 image: l3 (13x16) rows 0..12, two
            # zero separator rows, l4 (7x8) rows 15..21 in cols 0..7.
            # One 352-px conv tile covers both levels (junk rows 13/14 unused).
            HC, WC = 22, 16

            def load_34(xs):
                xp = xs.tile([128, 2, HC + 2, WC + 2], f32r, tag="xp34")
                for h in range(2):
                    nc.vector.memset(xp[:, h, 0:1, :].bitcast(f32), 0.0)
                    nc.vector.memset(xp[:, h, 14:16, :].bitcast(f32), 0.0)
                    nc.vector.memset(xp[:, h, 23:24, :].bitcast(f32), 0.0)
                    nc.vector.memset(xp[:, h, 1 : HC + 1, 0:1].bitcast(f32), 0.0)
                    nc.vector.memset(
                        xp[:, h, 1 : HC + 1, WC + 1 : WC + 2].bitcast(f32), 0.0
                    )
                    nc.vector.memset(xp[:, h, 16:23, 9:17].bitcast(f32), 0.0)
                for kh in range(2):
                    nc.gpsimd.dma_start(
                        out=xp[:, kh, 1:14, 1:17],
                        in_=feats[3][128 * kh : 128 * (kh + 1), :, :],
                    )
                    nc.gpsimd.dma_start(
                        out=xp[:, kh, 16:23, 1:9],
                        in_=feats[4][128 * kh : 128 * (kh + 1), :, :],
                    )
                return xp

            def smalls34_layer(xs, ys, st, pspool, holder, wt_t, pk_t):
                xp = holder[0]
                y = ys.tile([128, 2, HC, WC], f32, tag="y34")
                st3 = st.tile([128, 2, 1, 6], f32, tag="stats3")
                st4 = st.tile([128, 2, 7, 6], f32, tag="stats4")
                for mh in range(2):
                    ps = pspool.tile([128, 512], f32, tag=f"ps{mh}")
                    conv_tile(
                        ps[:, : HC * WC].rearrange("p (r w) -> p r w", r=HC),
                        wt_t, xp, None, ("tower", mh), 0, HC, width=WC,
                    )
                    nc.scalar.activation(
                        y[:, mh],
                        ps[:, : HC * WC].rearrange("p (r w) -> p r w", r=HC),
                        AF.Copy,
                    )
                    nc.vector.bn_stats(
                        out=st3[:, mh, 0, :],
                        in_=y[:, mh, 0:13, :].rearrange("p r w -> p (r w)"),
                    )
                    for i in range(7):
                        nc.vector.bn_stats(
                            out=st4[:, mh, i, :], in_=y[:, mh, 15 + i, 0:8]
                        )
                ab3 = gn_finalize(st, pspool, st3, pk_t, [(0, 13)], 16, "3")
                ab4 = gn_finalize(st, pspool, st4, pk_t, [(i, 1) for i in range(7)], 8, "4")
                xp_new = xs.tile([128, 2, HC + 2, WC + 2], f32r, tag="xp34")
                for h in range(2):
                    nc.vector.memset(xp_new[:, h, 0:1, :].bitcast(f32), 0.0)
                    nc.vector.memset(xp_new[:, h, 14:16, :].bitcast(f32), 0.0)
                    nc.vector.memset(xp_new[:, h, 23:24, :].bitcast(f32), 0.0)
                    nc.vector.memset(xp_new[:, h, 1 : HC + 1, 0:1].bitcast(f32), 0.0)
                    nc.vector.memset(
                        xp_new[:, h, 1 : HC + 1, WC + 1 : WC + 2].bitcast(f32), 0.0
                    )
                    nc.vector.memset(xp_new[:, h, 16:23, 9:17].bitcast(f32), 0.0)
                    nc.scalar.activation(
                        xp_new[:, h, 1:14, 1:17],
                        y[:, h, 0:13, :],
                        AF.Relu,
                        scale=ab3[:, h : h + 1],
                        bias=ab3[:, 2 + h : 3 + h],
                    )
                    nc.scalar.activation(
                        xp_new[:, h, 16:23, 1:9],
                        y[:, h, 15:22, 0:8],
                        AF.Relu,
                        scale=ab4[:, h : h + 1],
                        bias=ab4[:, 2 + h : 3 + h],
                    )
                holder[0] = xp_new

            def heads34(pspool, hpool, holder, wch_t, wbh_t):
                xp = holder[0]
                is_cls = wch_t is not None
                M = 16 if is_cls else 5
                wt_t = wch_t if is_cls else wbh_t
                ps = pspool.tile([128, 512], f32, tag="ps0")
                conv_tile(
                    ps[:M, : HC * WC].rearrange("p (r w) -> p r w", r=HC),
                    wt_t, xp, None, ("head", M), 0, HC, width=WC,
                )
                psv = ps[:, : HC * WC].rearrange("p (r w) -> p r w", r=HC)
                if is_cls:
                    hs = hpool.tile([16, HC, WC], f32, tag="hscls")
                    nc.vector.tensor_scalar_add(hs, psv[:16], hbc_t[:, 0:1])
                    nc.sync.dma_start(
                        out=out[0:16, OFFS[3] : OFFS[3] + 208],
                        in_=hs[:, 0:13, :].rearrange("p r w -> p (r w)"),
                    )
                    nc.sync.dma_start(
                        out=out[0:16, OFFS[4] : OFFS[4] + 56].rearrange(
                            "p (r w) -> p r w", r=7
                        ),
                        in_=hs[:, 15:22, 0:8],
                    )
                else:
                    hs = hpool.tile([5, HC, WC], f32, tag="hsbc")
                    hr = hpool.tile([5, HC, WC], f32, tag="hsraw")
                    nc.vector.tensor_copy(hr, psv[:5])
                    for lvl, rows, colw in ((3, slice(0, 13), 16), (4, slice(15, 22), 8)):
                        s = float(scales[lvl]) * float(STRIDES[lvl])
                        nc.vector.tensor_scalar(
                            hs[:4, rows, :colw], psv[:4, rows, :colw],
                            s, hbb_t[:, lvl : lvl + 1],
                            op0=mybir.AluOpType.mult, op1=mybir.AluOpType.add,
                        )
                        nc.vector.tensor_scalar_max(
                            hs[:4, rows, :colw], hs[:4, rows, :colw], 0.0
                        )
                    nc.sync.dma_start(
                        out=out[16:20, OFFS[3] : OFFS[3] + 208],
                        in_=hs[:4, 0:13, :].rearrange("p r w -> p (r w)"),
                    )
                    nc.sync.dma_start(
                        out=out[20:21, OFFS[3] : OFFS[3] + 208],
                        in_=hr[4:5, 0:13, :].rearrange("p r w -> p (r w)"),
                    )
                    nc.sync.dma_start(
                        out=out[16:20, OFFS[4] : OFFS[4] + 56].rearrange(
                            "p (r w) -> p r w", r=7
                        ),
                        in_=hs[:4, 15:22, 0:8],
                    )
                    nc.sync.dma_start(
                        out=out[20:21, OFFS[4] : OFFS[4] + 56].rearrange(
                            "p (r w) -> p r w", r=7
                        ),
                        in_=hr[4:5, 15:22, 0:8],
                    )

            # =====================================================
            # Phase A: levels 1-4, tower-major (weights loaded once/layer)
            # =====================================================
            import os as _os
            _phases = _os.environ.get("KPHASES", "AB")
            A_LEVELS = [1, 2]   # levels 3/4 ride along in phase B as filler
            B_LEVELS = [3, 4]
            # phase-B weight pool opened early: its SBUF never overlaps
            # phase A's pools, so B's first weight DMA can run during A
            wp0_pool = tc.tile_pool(name="wp0", bufs=2)
            wp0 = wp0_pool.__enter__()

            def load_levels(xs, levels):
                xps = {}
                for l in levels:
                    H, W = LEVELS[l]
                    xp = xs.tile([128, 2, H + 2, W + 2], f32r, tag=f"xp{l}")
                    memset_borders(xp, l)
                    for kh in range(2):
                        nc.gpsimd.dma_start(
                            out=xp[:, kh, 1 : H + 1, 1 : W + 1],
                            in_=feats[l][128 * kh : 128 * (kh + 1), :, :],
                        )
                    xps[l] = xp
                return xps

            def load_w(wp, st, t, L):
                wt_t = wp.tile([128, 2, 9, 2, 128], f32r, tag="w")
                # split by input-channel half: conv_tile consumes kh=0 taps
                # first, so the first matmuls start after half the DMA
                src = wts[(t, L)][:, :].rearrange(
                    "p (a b c d) -> p a b c d", a=2, b=9, c=2
                )
                for kh in range(2):
                    nc.gpsimd.dma_start(
                        out=wt_t[:, kh : kh + 1], in_=src[:, kh : kh + 1]
                    )
                pk_t = st.tile([128, 2, 3], f32, tag="pk")
                nc.sync.dma_start(
                    out=pk_t, in_=pks[(t, L)][:, :].rearrange("p (a b) -> p a b", a=2)
                )
                return wt_t, pk_t

            def load_head_w(wp, t):
                if t == "c":
                    wh = wp.tile([128, 2, 9, 16], f32r, tag="wh")
                    src_ap = wch[:, :].rearrange("p (a b m) -> p a b m", a=2, b=9)
                else:
                    wh = wp.tile([128, 2, 9, 5], f32r, tag="wh")
                    src_ap = wbh[:, :].rearrange("p (a b m) -> p a b m", a=2, b=9)
                nc.gpsimd.dma_start(out=wh, in_=src_ap)
                return wh

            def smalls_layer(xs, ys, st, pspool, xps, wt_t, pk_t, levels):
                held = {}
                for l in levels:
                    H, W = LEVELS[l]
                    tiles = _tiles_for(l)
                    y = ys.tile([128, 2, H, W], f32, tag=f"y{l}")
                    stats_t = st.tile([128, 2, len(tiles), 6], f32, tag=f"stats{l}")
                    for ti, (r0, R) in enumerate(tiles):
                        N = R * W
                        for mh in range(2):
                            ps = pspool.tile([128, 512], f32, tag=f"ps{mh}")
                            conv_tile(
                                ps[:, :N].rearrange("p (r w) -> p r w", r=R),
                                wt_t, xps[l], l, ("tower", mh), r0, R,
                            )
                            nc.scalar.activation(
                                y[:, mh, r0 : r0 + R, :],
                                ps[:, :N].rearrange("p (r w) -> p r w", r=R),
                                AF.Copy,
                            )
                            nc.vector.bn_stats(
                                out=stats_t[:, mh, ti, :],
                                in_=y[:, mh, r0 : r0 + R, :].rearrange(
                                    "p r w -> p (r w)"
                                ),
                            )
                    held[l] = (y, stats_t, tiles)
                for l in levels:
                    H, W = LEVELS[l]
                    y, stats_t, tiles = held[l]
                    ab = gn_finalize(st, pspool, stats_t, pk_t, tiles, W)
                    xp_new = xs.tile([128, 2, H + 2, W + 2], f32r, tag=f"xp{l}")
                    memset_borders(xp_new, l)
                    for h in range(2):
                        nc.scalar.activation(
                            xp_new[:, h, 1 : H + 1, 1 : W + 1],
                            y[:, h],
                            AF.Relu,
                            scale=ab[:, h : h + 1],
                            bias=ab[:, 2 + h : 3 + h],
                        )
                    xps[l] = xp_new

            if "A" in _phases:
                with tc.tile_pool(name="wp", bufs=2) as wp, \
                     tc.tile_pool(name="xs", bufs=2) as xs, \
                     tc.tile_pool(name="ys", bufs=1) as ys, \
                     tc.tile_pool(name="st", bufs=2) as st, \
                     tc.tile_pool(name="hp", bufs=4) as hp, \
                     tc.tile_pool(name="ps", bufs=2, space="PSUM") as pspool:
                    xps = load_levels(xs, A_LEVELS)
                    pre_w = None
                    for t in ("c", "b"):
                        for L in range(NUM_CONVS):
                            if L == 0 and pre_w is not None:
                                wt_t, pk_t = pre_w
                            else:
                                wt_t, pk_t = load_w(wp, st, t, L)
                            smalls_layer(xs, ys, st, pspool, xps, wt_t, pk_t, A_LEVELS)
                        if t == "c":
                            # prefetch bbox inputs + first weights during heads
                            xps_b = load_levels(xs, A_LEVELS)
                            pre_w = load_w(wp, st, "b", 0)
                        wh = load_head_w(wp, t)
                        for l in A_LEVELS:
                            if t == "c":
                                head_convs(xps[l], l, pspool, hp, wh, None)
                            else:
                                head_convs(xps[l], l, pspool, hp, None, wh)
                        if t == "c":
                            xps = xps_b

            # =====================================================
            # Phase B: level 0, streamed through DRAM scratch.
            # Layer 0 runs BOTH towers off the single initial load; the
            # bbox path restarts later from its parked DRAM conv output.
            # =====================================================
            H, W = LEVELS[0]
            tiles0 = _tiles_for(0)
            if "B" in _phases:
                wp = wp0
                with tc.tile_pool(name="x0", bufs=1) as x0p, \
                     tc.tile_pool(name="st0", bufs=2) as st, \
                     tc.tile_pool(name="stg", bufs=2) as stg, \
                     tc.tile_pool(name="rfl", bufs=2, space="SBUF") as rfl, \
                     tc.tile_pool(name="hp0", bufs=2) as hp, \
                     tc.tile_pool(name="xs34", bufs=2) as xs34, \
                     tc.tile_pool(name="ys34", bufs=1) as ys34, \
                     tc.tile_pool(name="dr", bufs=2, space="DRAM") as drp, \
                     tc.tile_pool(name="ps0", bufs=2, space="PSUM") as pspool:
                    # weights first: the first conv only needs wc0 + a few
                    # input rows, so don't queue 13MB of image ahead of it
                    wc0, pc0 = load_w(wp, st, "c", 0)
                    xp = x0p.tile([128, 2, H + 2, W + 2], f32r, tag="xp0")
                    memset_borders(xp, 0)
                    # chunked load: row-range deps let layer-0 convs start
                    # as soon as their input rows land
                    for r in range(0, H, F0_REFILL_ROWS):
                        RR = min(F0_REFILL_ROWS, H - r)
                        for kh in range(2):
                            nc.gpsimd.dma_start(
                                out=xp[:, kh, r + 1 : r + RR + 1, 1 : W + 1],
                                in_=feats[0][128 * kh : 128 * (kh + 1), r : r + RR, :],
                            )

                    def f0_conv_layer(wt_t, yraw, stats_t):
                        for ti, (r0, R) in enumerate(tiles0):
                            N = R * W
                            sg = stg.tile([128, 2, 512], f32, tag="sg")
                            for mh in range(2):
                                ps = pspool.tile([128, 512], f32, tag=f"ps{mh}")
                                conv_tile(
                                    ps[:, :N].rearrange("p (r w) -> p r w", r=R),
                                    wt_t, xp, 0, ("tower", mh), r0, R,
                                )
                                nc.scalar.activation(sg[:, mh, :N], ps[:, :N], AF.Copy)
                                nc.vector.bn_stats(
                                    out=stats_t[:, mh, ti, :], in_=sg[:, mh, :N]
                                )
                            nc.sync.dma_start(
                                out=yraw[:, :, r0 * W : r0 * W + N], in_=sg[:, :, :N]
                            )

                    def f0_refill_chunk(yraw, ab, r, RR=None):
                        RR = min(RR or F0_REFILL_ROWS, H - r)
                        rt = rfl.tile([128, 2, F0_REFILL_ROWS, W], f32, tag="rt")
                        nc.sync.dma_start(
                            out=rt[:, :, :RR, :].rearrange("p a r w -> p a (r w)"),
                            in_=yraw[:, :, r * W : (r + RR) * W],
                        )
                        for h in range(2):
                            nc.scalar.activation(
                                xp[:, h, r + 1 : r + RR + 1, 1 : W + 1],
                                rt[:, h, :RR, :],
                                AF.Relu,
                                scale=ab[:, h : h + 1],
                                bias=ab[:, 2 + h : 3 + h],
                            )

                    def f0_refill(yraw, ab):
                        # small leading chunk: the next layer's first conv
                        # tile only needs ~5 rows, so publish them early
                        f0_refill_chunk(yraw, ab, 0, 4)
                        r = 4
                        while r < H:
                            f0_refill_chunk(yraw, ab, r)
                            r += F0_REFILL_ROWS

                    # levels 3/4 ride along as PE filler between f0 layers
                    xp34h = [load_34(xs34)]

                    # layer 0, both towers, off the pristine input
                    yraw_c = drp.tile([128, 2, H * W], f32, tag="yrc")
                    stats_c = st.tile([128, 2, len(tiles0), 6], f32, tag="stats0")
                    f0_conv_layer(wc0, yraw_c, stats_c)
                    wb0, pb0 = load_w(wp, st, "b", 0)
                    pb0b = st.tile([128, 2, 3], f32, tag="pkb")
                    nc.vector.tensor_copy(pb0b, pb0)
                    yraw_b = drp.tile([128, 2, H * W], f32, tag="yrb")
                    stats_b = st.tile([128, 2, len(tiles0), 6], f32, tag="stats0b")
                    f0_conv_layer(wb0, yraw_b, stats_b)
                    smalls34_layer(xs34, ys34, st, pspool, xp34h, wc0, pc0)
                    ab_b = gn_finalize(st, pspool, stats_b, pb0b, tiles0, W, "b")
                    ab_c = gn_finalize(st, pspool, stats_c, pc0, tiles0, W, "0")
                    f0_refill(yraw_c, ab_c)

                    for t in ("c", "b"):
                        lo = 1 if t == "c" else 0
                        for L in range(lo, NUM_CONVS):
                            wt_t, pk_t = load_w(wp, st, t, L)
                            if L >= 1:
                                yraw = drp.tile([128, 2, H * W], f32, tag="yrc")
                                stats_t = st.tile(
                                    [128, 2, len(tiles0), 6], f32, tag="stats0"
                                )
                                f0_conv_layer(wt_t, yraw, stats_t)
                            smalls34_layer(
                                xs34, ys34, st, pspool, xp34h, wt_t, pk_t
                            )
                            if L >= 1:
                                ab = gn_finalize(
                                    st, pspool, stats_t, pk_t, tiles0, W, "0"
                                )
                                f0_refill(yraw, ab)
                        wh = load_head_w(wp, t)
                        if t == "c":
                            # bbox path restarts from the parked layer-0 output;
                            # its refill chunks interleave with cls head tiles
                            # right after each chunk's last WAR reader
                            inter = {}
                            nt0 = len(tiles0)
                            for k, r in enumerate(range(0, H, F0_REFILL_ROWS)):
                                last_reader = min(
                                    nt0 - 1, (r + F0_REFILL_ROWS) // ROWS[0]
                                )
                                inter.setdefault(last_reader, []).append(r)

                            def mk(rs):
                                return lambda: [
                                    f0_refill_chunk(yraw_b, ab_b, r) for r in rs
                                ]

                            inter = {ti: mk(rs) for ti, rs in inter.items()}
                            head_convs(xp, 0, pspool, hp, wh, None, interleave=inter)
                            heads34(pspool, hp, xp34h, wh, None)
                            # fresh l3/l4 inputs for the bbox tower
                            xp34h = [load_34(xs34)]
                        else:
                            head_convs(xp, 0, pspool, hp, None, wh)
                            heads34(pspool, hp, xp34h, None, wh)

            wp0_pool.__exit__(None, None, None)

    nc.compile()
    return nc


# ---------------------------------------------------------------- entry
_CACHE = {}


def kernel(f0, f1, f2, f3, f4, params):
    from concourse.bass_utils import run_bass_kernel_spmd

    feats = [_np(f0), _np(f1), _np(f2), _np(f3), _np(f4)]
    scales = _np(params["scales"])

    wmap = {}
    for t, key in (("c", "cls_tower"), ("b", "bbox_tower")):
        for L, (W, b, g, be) in enumerate(params[key]):
            wmap[f"w{t}{L}"] = _prep_tower_w(W)
            pk = np.stack([_np(g), _np(be), _np(b)], axis=1)  # [256, 3]
            wmap[f"p{t}{L}"] = np.ascontiguousarray(
                pk.reshape(2, 128, 3).transpose(1, 0, 2).reshape(128, 6)
            )
    wmap["wch"], _ = _prep_head_w([params["cls_W"]])
    wmap["wbh"], _ = _prep_head_w([params["box_W"], params["ctr_W"]])
    wmap["hbc"] = _np(params["cls_b"]).reshape(16, 1)
    hbb = np.zeros((4, 5), np.float32)
    for l in range(5):
        hbb[:, l] = _np(params["box_b"]) * float(scales[l]) * float(STRIDES[l])
    wmap["hbb"] = hbb
    ind16 = np.zeros((128, 16), np.float32)
    for g in range(16):
        ind16[8 * g : 8 * (g + 1), g] = 0.125
    wmap["ind16"] = ind16
    bc16 = np.zeros((16, 128), np.float32)
    for g in range(16):
        bc16[g, 8 * g : 8 * (g + 1)] = 1.0
    wmap["bc16"] = bc16

    key = scales.tobytes()
    if key not in _CACHE:
        _CACHE[key] = build_program(scales)
    nc = _CACHE[key]

    in_maps = []
    for b in range(NCORES):
        m = {f"f{l}": feats[l][b] for l in range(5)}
        m.update(wmap)
        in_maps.append(m)

    # the axon/PJRT execute occasionally faults transiently
    # (NRT_EXEC_UNIT_UNRECOVERABLE); retry a couple of times
    import time

    last = None
    for attempt in range(3):
        try:
            res = run_bass_kernel_spmd(nc, in_maps, core_ids=list(range(NCORES)))
            break
        except Exception as e:  # noqa: BLE001
            last = e
            if attempt == 2:
                raise
            time.sleep(5.0)
    out = np.stack([res.results[b]["out"] for b in range(NCORES)], axis=0)
    return out, _locations()


# revision 52
# speedup vs baseline: 1.0027x; 1.0027x over previous
"""FCOS head (nms_detection) Trainium2 Bass kernel.

Strategy: data-parallel over batch across 8 NeuronCores (1 image/core,
weights replicated).  Per core, each 3x3 SAME conv is computed as 18
accumulating float32r matmuls (9 taps x 2 input-channel halves) per
<=512-pixel output tile, reading from a zero-padded SBUF activation
image.  GroupNorm statistics come from bn_stats/bn_aggr per channel,
then two tiny exact-fp32 matmuls do the cross-partition group reduce
and broadcast; normalize+ReLU is a single fused scalar-engine
activation (Relu(A*x+B)) per channel-half.  Level 0 (100x128) streams
conv outputs through a DRAM scratch buffer (its x and y don't both fit
in SBUF); levels 1-4 stay SBUF-resident.  Head convs (cls / box+ctr)
use the same tap-matmul scheme with M=16 / M=5.
"""

import math

import numpy as np

# ---------------------------------------------------------------- constants
IN_CH = 256
NUM_CLASSES = 16
NUM_CONVS = 4
STRIDES = (8, 16, 32, 64, 128)
GN_EPS = 1e-5
NCORES = 8

# (H, W) per level
LEVELS = [(100, 128), (50, 64), (25, 32), (13, 16), (7, 8)]
# output-row chunk per level (rows*W <= 512).  Chunk sizes may be unequal;
# gn stats aggregate per size-group and combine with host-known weights.
ROWS = [4, 8, 16, 13, 7]
OFFS = [0, 12800, 16000, 16800, 17008]
TOTAL_LOCS = 17064

F0_REFILL_ROWS = 8  # rows per refill chunk for level 0


def _np(x):
    return np.ascontiguousarray(np.asarray(x), dtype=np.float32)


def _tiles_for(level):
    H, _ = LEVELS[level]
    R = ROWS[level]
    out = []
    r = 0
    while r < H:
        out.append((r, min(R, H - r)))
        r += R
    return out


def _prep_tower_w(W):
    # W [O=256, I=256, 3, 3] -> [p=128, kh=2, tap=9, mh=2, m=128] flattened
    W = _np(W).reshape(2, 128, 2, 128, 3, 3)  # [mh, m, kh, p, dy, dx]
    wt = np.transpose(W, (3, 2, 4, 5, 0, 1))  # [p, kh, dy, dx, mh, m]
    return np.ascontiguousarray(wt.reshape(128, 2 * 9 * 2 * 128))


def _prep_head_w(Ws):
    # list of [o_i, 256, 3, 3] stacked on o -> [p, kh, tap, m_total]
    W = np.concatenate([_np(w) for w in Ws], axis=0)  # [M, 256, 3, 3]
    M = W.shape[0]
    W = W.reshape(M, 2, 128, 3, 3)  # [m, kh, p, dy, dx]
    wt = np.transpose(W, (2, 1, 3, 4, 0))  # [p, kh, dy, dx, m]
    return np.ascontiguousarray(wt.reshape(128, 2 * 9 * M)), M


def _locations():
    locs = []
    for l, (h, w) in enumerate(LEVELS):
        s = STRIDES[l]
        sx = np.arange(0, w * s, s, dtype=np.float32)
        sy = np.arange(0, h * s, s, dtype=np.float32)
        yy, xx = np.meshgrid(sy, sx, indexing="ij")
        locs.append(np.stack([xx.reshape(-1), yy.reshape(-1)], axis=1) + s // 2)
    return np.concatenate(locs, axis=0).astype(np.float32)


# ---------------------------------------------------------------- program
def build_program(scales):
    import concourse.bacc as bacc
    import concourse.mybir as mybir
    import concourse.tile as tile

    f32 = mybir.dt.float32
    f32r = mybir.dt.float32r
    AF = mybir.ActivationFunctionType

    nc = bacc.Bacc(trn_type="TRN2", num_swdge_queues=4)

    feats = [
        nc.declare_dram_parameter(f"f{l}", [IN_CH, H, W], f32, isOutput=False)
        for l, (H, W) in enumerate(LEVELS)
    ]
    wts = {}
    pks = {}
    for t in ("c", "b"):
        for L in range(NUM_CONVS):
            wts[(t, L)] = nc.declare_dram_parameter(
                f"w{t}{L}", [128, 4608], f32, isOutput=False
            )
            pks[(t, L)] = nc.declare_dram_parameter(
                f"p{t}{L}", [128, 6], f32, isOutput=False
            )
    wch = nc.declare_dram_parameter("wch", [128, 288], f32, isOutput=False)
    wbh = nc.declare_dram_parameter("wbh", [128, 90], f32, isOutput=False)
    hbc = nc.declare_dram_parameter("hbc", [16, 1], f32, isOutput=False)
    hbb = nc.declare_dram_parameter("hbb", [4, 5], f32, isOutput=False)
    ind16_d = nc.declare_dram_parameter("ind16", [128, 16], f32, isOutput=False)
    bc16_d = nc.declare_dram_parameter("bc16", [16, 128], f32, isOutput=False)
    out = nc.declare_dram_parameter("out", [21, TOTAL_LOCS], f32, isOutput=True)

    with tile.TileContext(nc) as tc:
        # ---- constants that live for the whole kernel
        with tc.tile_pool(name="konst", bufs=1) as kp:
            ind16 = kp.tile([128, 16], f32)
            nc.sync.dma_start(out=ind16, in_=ind16_d[:, :])
            bc16 = kp.tile([16, 128], f32)
            nc.sync.dma_start(out=bc16, in_=bc16_d[:, :])
            hbc_t = kp.tile([16, 1], f32)
            nc.sync.dma_start(out=hbc_t, in_=hbc[:, :])
            hbb_t = kp.tile([4, 5], f32)
            nc.sync.dma_start(out=hbb_t, in_=hbb[:, :])
            epst = kp.tile([128, 1], f32)
            nc.vector.memset(epst, GN_EPS)

            def conv_tile(ps, wt_t, xp, level, kind, r0, R, width=None):
                """Accumulate 3x3 conv for output rows [r0, r0+R) into psum.

                kind: ("tower", mh) or ("head", M) selecting lhsT slice.
                wt_t viewed as [128, 2, 9, X]."""
                W = width if width is not None else LEVELS[level][1]
                first = True
                for kh in range(2):
                    for tap in range(9):
                        dy, dx = tap // 3, tap % 3
                        rhs = xp[:, kh, r0 + dy : r0 + dy + R, dx : dx + W]
                        if kind[0] == "tower":
                            lhsT = wt_t[:, kh, tap, kind[1], :]
                        else:
                            lhsT = wt_t[:, kh, tap, : kind[1]]
                        nc.tensor.matmul(
                            ps,
                            lhsT,
                            rhs,
                            start=first,
                            stop=(kh == 1 and tap == 8),
                        )
                        first = False

            def gn_finalize(spool, pspool, stats_t, pk_t, tile_list, W, tagsfx=""):
                """Per-channel chunk stats -> per-channel affine (A,B) [128,4].

                Chunks may have unequal pixel counts: bn_aggr is unweighted, so
                aggregate per size-group and combine with host-known weights."""
                Ns = [R * W for (_, R) in tile_list]
                groups = []  # (ti_start, ti_end, chunk_px)
                for ti, n in enumerate(Ns):
                    if groups and groups[-1][2] == n:
                        groups[-1][1] = ti + 1
                    else:
                        groups.append([ti, ti + 1, n])
                total = float(sum(Ns))
                # me cols: mean0 e2_0 mean1 e2_1 (bias-free mean and E[x^2])
                me = spool.tile([128, 4], f32, tag="me" + tagsfx)
                sq = spool.tile([128, 1], f32, tag="sq" + tagsfx)
                mv = spool.tile([128, 2], f32, tag="mv" + tagsfx)
                acc = spool.tile([128, 2], f32, tag="macc" + tagsfx)
                for h in range(2):
                    if len(groups) == 1:
                        nc.vector.bn_aggr(out=mv, in_=stats_t[:, h])
                        nc.vector.tensor_copy(me[:, 2 * h : 2 * h + 1], mv[:, 0:1])
                        nc.vector.tensor_mul(sq, mv[:, 0:1], mv[:, 0:1])
                        nc.vector.tensor_add(
                            me[:, 2 * h + 1 : 2 * h + 2], mv[:, 1:2], sq
                        )
                    else:
                        first = True
                        for (s, e, n) in groups:
                            w_g = (e - s) * n / total
                            nc.vector.bn_aggr(out=mv, in_=stats_t[:, h, s:e])
                            # e2_g = var + mean^2 ; acc += w_g * [mean, e2_g]
                            nc.vector.tensor_mul(sq, mv[:, 0:1], mv[:, 0:1])
                            nc.vector.tensor_add(mv[:, 1:2], mv[:, 1:2], sq)
                            nc.vector.tensor_scalar_mul(mv, mv, w_g)
                            if first:
                                nc.vector.tensor_copy(acc, mv)
                                first = False
                            else:
                                nc.vector.tensor_add(acc, acc, mv)
                        nc.vector.tensor_copy(me[:, 2 * h : 2 * h + 1], acc[:, 0:1])
                        nc.vector.tensor_copy(
                            me[:, 2 * h + 1 : 2 * h + 2], acc[:, 1:2]
                        )
                # mbuf cols: m0 s0 m1 s1 ; m = mean + b, s = e2 + b*(mean + m)
                mbuf = spool.tile([128, 4], f32, tag="mbuf" + tagsfx)
                for h in range(2):
                    nc.vector.tensor_add(
                        mbuf[:, 2 * h : 2 * h + 1], me[:, 2 * h : 2 * h + 1],
                        pk_t[:, h, 2:3],
                    )
                    nc.vector.tensor_add(
                        sq, me[:, 2 * h : 2 * h + 1], mbuf[:, 2 * h : 2 * h + 1]
                    )
                    nc.vector.tensor_mul(sq, sq, pk_t[:, h, 2:3])
                    nc.vector.tensor_add(
                        mbuf[:, 2 * h + 1 : 2 * h + 2],
                        me[:, 2 * h + 1 : 2 * h + 2], sq,
                    )
                gps = pspool.tile([128, 4], f32, tag="pstat")
                nc.tensor.matmul(gps[:16, :], ind16, mbuf, start=True, stop=True)
                gsb = spool.tile([16, 4], f32, tag="gsb" + tagsfx)
                nc.scalar.activation(gsb, gps[:16, :], AF.Copy)
                bps = pspool.tile([128, 4], f32, tag="pstat")
                nc.tensor.matmul(bps, bc16, gsb, start=True, stop=True)
                bsb = spool.tile([128, 4], f32, tag="bsb" + tagsfx)
                nc.scalar.activation(bsb, bps, AF.Copy)
                # var = E2 - mu^2 ; rstd = 1/sqrt(var+eps)
                ab = spool.tile([128, 4], f32, tag="ab" + tagsfx)  # A0 A1 B0 B1
                var = spool.tile([128, 2], f32, tag="var" + tagsfx)
                for h in range(2):
                    nc.vector.tensor_mul(sq, bsb[:, 2 * h : 2 * h + 1], bsb[:, 2 * h : 2 * h + 1])
                    nc.vector.tensor_sub(var[:, h : h + 1], bsb[:, 2 * h + 1 : 2 * h + 2], sq)
                    nc.scalar.activation(
                        var[:, h : h + 1], var[:, h : h + 1], AF.Sqrt, bias=epst[:, 0:1]
                    )
                    nc.vector.reciprocal(var[:, h : h + 1], var[:, h : h + 1])
                    nc.vector.tensor_mul(
                        ab[:, h : h + 1], var[:, h : h + 1], pk_t[:, h, 0:1]
                    )
                    # B = beta + A*(bias - mu)
                    nc.vector.tensor_sub(sq, pk_t[:, h, 2:3], bsb[:, 2 * h : 2 * h + 1])
                    nc.vector.tensor_mul(sq, ab[:, h : h + 1], sq)
                    nc.vector.tensor_add(ab[:, 2 + h : 3 + h], pk_t[:, h, 1:2], sq)
                return ab

            def memset_borders(xp, level):
                H, W = LEVELS[level]
                for h in range(2):
                    nc.vector.memset(xp[:, h, 0:1, :].bitcast(f32), 0.0)
                    nc.vector.memset(xp[:, h, H + 1 : H + 2, :].bitcast(f32), 0.0)
                    nc.vector.memset(xp[:, h, 1 : H + 1, 0:1].bitcast(f32), 0.0)
                    nc.vector.memset(
                        xp[:, h, 1 : H + 1, W + 1 : W + 2].bitcast(f32), 0.0
                    )

            def head_convs(xp, level, pspool, hpool, wch_t, wbh_t, interleave=None):
                """cls head if wch_t else box+ctr head, from padded tower out.

                interleave: optional {tile_idx: callable} of filler work to
                emit between head tiles (keeps engine FIFOs pipelined)."""
                H, W = LEVELS[level]
                off = OFFS[level]
                for ti, (r0, R) in enumerate(_tiles_for(level)):
                    N = R * W
                    cols = slice(off + r0 * W, off + r0 * W + N)
                    if wch_t is not None:
                        ps = pspool.tile([128, 512], f32, tag=f"ps{ti % 2}")
                        conv_tile(
                            ps[:16, :N].rearrange("p (r w) -> p r w", r=R),
                            wch_t, xp, level, ("head", 16), r0, R,
                        )
                        hs = hpool.tile([16, 512], f32, tag="hscls")
                        nc.vector.tensor_scalar_add(hs[:, :N], ps[:16, :N], hbc_t[:, 0:1])
                        nc.sync.dma_start(out=out[0:16, cols], in_=hs[:, :N])
                    else:
                        s = float(scales[level]) * float(STRIDES[level])
                        ps = pspool.tile([128, 512], f32, tag=f"ps{ti % 2}")
                        conv_tile(
                            ps[:5, :N].rearrange("p (r w) -> p r w", r=R),
                            wbh_t, xp, level, ("head", 5), r0, R,
                        )
                        hs = hpool.tile([5, 512], f32, tag="hsbc")
                        hr = hpool.tile([5, 512], f32, tag="hsraw")
                        nc.vector.tensor_copy(hr[:, :N], ps[:5, :N])
                        # box rows: relu(s*t*x + b*s*t) = s*t*relu(x + b), s,t>0
                        nc.vector.tensor_scalar(
                            hs[:4, :N], ps[:4, :N],
                            s, hbb_t[:, level : level + 1],
                            op0=mybir.AluOpType.mult, op1=mybir.AluOpType.add,
                        )
                        nc.vector.tensor_scalar_max(hs[:4, :N], hs[:4, :N], 0.0)
                        nc.sync.dma_start(out=out[16:20, cols], in_=hs[:4, :N])
                        nc.sync.dma_start(out=out[20:21, cols], in_=hr[4:5, :N])
                    if interleave and ti in interleave:
                        interleave[ti]()

            # ---- combined level 3+4 image: l3 (13x16) rows 0..12, two
            # zero separator rows, l4 (7x8) rows 15..21 in cols 0..7.
            # One 352-px conv tile covers both levels (junk rows 13/14 unused).
            HC, WC = 22, 16

            def load_34(xs):
                xp = xs.tile([128, 2, HC + 2, WC + 2], f32r, tag="xp34")
                for h in range(2):
                    nc.vector.memset(xp[:, h, 0:1, :].bitcast(f32), 0.0)
                    nc.vector.memset(xp[:, h, 14:16, :].bitcast(f32), 0.0)
                    nc.vector.memset(xp[:, h, 23:24, :].bitcast(f32), 0.0)
                    nc.vector.memset(xp[:, h, 1 : HC + 1, 0:1].bitcast(f32), 0.0)
                    nc.vector.memset(
                        xp[:, h, 1 : HC + 1, WC + 1 : WC + 2].bitcast(f32), 0.0
                    )
                    nc.vector.memset(xp[:, h, 16:23, 9:17].bitcast(f32), 0.0)
                for kh in range(2):
                    nc.gpsimd.dma_start(
                        out=xp[:, kh, 1:14, 1:17],
                        in_=feats[3][128 * kh : 128 * (kh + 1), :, :],
                    )
                    nc.gpsimd.dma_start(
                        out=xp[:, kh, 16:23, 1:9],
                        in_=feats[4][128 * kh : 128 * (kh + 1), :, :],
                    )
                return xp

            def smalls34_layer(xs, ys, st, pspool, holder, wt_t, pk_t):
                xp = holder[0]
                y = ys.tile([128, 2, HC, WC], f32, tag="y34")
                st3 = st.tile([128, 2, 1, 6], f32, tag="stats3")
                st4 = st.tile([128, 2, 7, 6], f32, tag="stats4")
                for mh in range(2):
                    ps = pspool.tile([128, 512], f32, tag=f"ps{mh}")
                    conv_tile(
                        ps[:, : HC * WC].rearrange("p (r w) -> p r w", r=HC),
                        wt_t, xp, None, ("tower", mh), 0, HC, width=WC,
                    )
                    nc.scalar.activation(
                        y[:, mh],
                        ps[:, : HC * WC].rearrange("p (r w) -> p r w", r=HC),
                        AF.Copy,
                    )
                    nc.vector.bn_stats(
                        out=st3[:, mh, 0, :],
                        in_=y[:, mh, 0:13, :].rearrange("p r w -> p (r w)"),
                    )
                    for i in range(7):
                        nc.vector.bn_stats(
                            out=st4[:, mh, i, :], in_=y[:, mh, 15 + i, 0:8]
                        )
                ab3 = gn_finalize(st, pspool, st3, pk_t, [(0, 13)], 16, "3")
                ab4 = gn_finalize(st, pspool, st4, pk_t, [(i, 1) for i in range(7)], 8, "4")
                xp_new = xs.tile([128, 2, HC + 2, WC + 2], f32r, tag="xp34")
                for h in range(2):
                    nc.vector.memset(xp_new[:, h, 0:1, :].bitcast(f32), 0.0)
                    nc.vector.memset(xp_new[:, h, 14:16, :].bitcast(f32), 0.0)
                    nc.vector.memset(xp_new[:, h, 23:24, :].bitcast(f32), 0.0)
                    nc.vector.memset(xp_new[:, h, 1 : HC + 1, 0:1].bitcast(f32), 0.0)
                    nc.vector.memset(
                        xp_new[:, h, 1 : HC + 1, WC + 1 : WC + 2].bitcast(f32), 0.0
                    )
                    nc.vector.memset(xp_new[:, h, 16:23, 9:17].bitcast(f32), 0.0)
                    nc.scalar.activation(
                        xp_new[:, h, 1:14, 1:17],
                        y[:, h, 0:13, :],
                        AF.Relu,
                        scale=ab3[:, h : h + 1],
                        bias=ab3[:, 2 + h : 3 + h],
                    )
                    nc.scalar.activation(
                        xp_new[:, h, 16:23, 1:9],
                        y[:, h, 15:22, 0:8],
                        AF.Relu,
                        scale=ab4[:, h : h + 1],
                        bias=ab4[:, 2 + h : 3 + h],
                    )
                holder[0] = xp_new

            def heads34(pspool, hpool, holder, wch_t, wbh_t):
                xp = holder[0]
                is_cls = wch_t is not None
                M = 16 if is_cls else 5
                wt_t = wch_t if is_cls else wbh_t
                ps = pspool.tile([128, 512], f32, tag="ps0")
                conv_tile(
                    ps[:M, : HC * WC].rearrange("p (r w) -> p r w", r=HC),
                    wt_t, xp, None, ("head", M), 0, HC, width=WC,
                )
                psv = ps[:, : HC * WC].rearrange("p (r w) -> p r w", r=HC)
                if is_cls:
                    hs = hpool.tile([16, HC, WC], f32, tag="hscls")
                    nc.vector.tensor_scalar_add(hs, psv[:16], hbc_t[:, 0:1])
                    nc.sync.dma_start(
                        out=out[0:16, OFFS[3] : OFFS[3] + 208],
                        in_=hs[:, 0:13, :].rearrange("p r w -> p (r w)"),
                    )
                    nc.sync.dma_start(
                        out=out[0:16, OFFS[4] : OFFS[4] + 56].rearrange(
                            "p (r w) -> p r w", r=7
                        ),
                        in_=hs[:, 15:22, 0:8],
                    )
                else:
                    hs = hpool.tile([5, HC, WC], f32, tag="hsbc")
                    hr = hpool.tile([5, HC, WC], f32, tag="hsraw")
                    nc.vector.tensor_copy(hr, psv[:5])
                    for lvl, rows, colw in ((3, slice(0, 13), 16), (4, slice(15, 22), 8)):
                        s = float(scales[lvl]) * float(STRIDES[lvl])
                        nc.vector.tensor_scalar(
                            hs[:4, rows, :colw], psv[:4, rows, :colw],
                            s, hbb_t[:, lvl : lvl + 1],
                            op0=mybir.AluOpType.mult, op1=mybir.AluOpType.add,
                        )
                        nc.vector.tensor_scalar_max(
                            hs[:4, rows, :colw], hs[:4, rows, :colw], 0.0
                        )
                    nc.sync.dma_start(
                        out=out[16:20, OFFS[3] : OFFS[3] + 208],
                        in_=hs[:4, 0:13, :].rearrange("p r w -> p (r w)"),
                    )
                    nc.sync.dma_start(
                        out=out[20:21, OFFS[3] : OFFS[3] + 208],
                        in_=hr[4:5, 0:13, :].rearrange("p r w -> p (r w)"),
                    )
                    nc.sync.dma_start(
                        out=out[16:20, OFFS[4] : OFFS[4] + 56].rearrange(
                            "p (r w) -> p r w", r=7
                        ),
                        in_=hs[:4, 15:22, 0:8],
                    )
                    nc.sync.dma_start(
                        out=out[20:21, OFFS[4] : OFFS[4] + 56].rearrange(
                            "p (r w) -> p r w", r=7
                        ),
                        in_=hr[4:5, 15:22, 0:8],
                    )

            # =====================================================
            # Phase A: levels 1-4, tower-major (weights loaded once/layer)
            # =====================================================
            import os as _os
            _phases = _os.environ.get("KPHASES", "AB")
            A_LEVELS = [1, 2]   # levels 3/4 ride along in phase B as filler
            B_LEVELS = [3, 4]
            # phase-B weight pool opened early: its SBUF never overlaps
            # phase A's pools, so B's first weight DMA can run during A
            wp0_pool = tc.tile_pool(name="wp0", bufs=2)
            wp0 = wp0_pool.__enter__()

            def load_levels(xs, levels):
                xps = {}
                for l in levels:
                    H, W = LEVELS[l]
                    xp = xs.tile([128, 2, H + 2, W + 2], f32r, tag=f"xp{l}")
                    memset_borders(xp, l)
                    for kh in range(2):
                        nc.gpsimd.dma_start(
                            out=xp[:, kh, 1 : H + 1, 1 : W + 1],
                            in_=feats[l][128 * kh : 128 * (kh + 1), :, :],
                        )
                    xps[l] = xp
                return xps

            def load_w(wp, st, t, L):
                wt_t = wp.tile([128, 2, 9, 2, 128], f32r, tag="w")
                # split by input-channel half: conv_tile consumes kh=0 taps
                # first, so the first matmuls start after half the DMA
                src = wts[(t, L)][:, :].rearrange(
                    "p (a b c d) -> p a b c d", a=2, b=9, c=2
                )
                for kh in range(2):
                    nc.gpsimd.dma_start(
                        out=wt_t[:, kh : kh + 1], in_=src[:, kh : kh + 1]
                    )
                pk_t = st.tile([128, 2, 3], f32, tag="pk")
                nc.sync.dma_start(
                    out=pk_t, in_=pks[(t, L)][:, :].rearrange("p (a b) -> p a b", a=2)
                )
                return wt_t, pk_t

            def load_head_w(wp, t):
                if t == "c":
                    wh = wp.tile([128, 2, 9, 16], f32r, tag="wh")
                    src_ap = wch[:, :].rearrange("p (a b m) -> p a b m", a=2, b=9)
                else:
                    wh = wp.tile([128, 2, 9, 5], f32r, tag="wh")
                    src_ap = wbh[:, :].rearrange("p (a b m) -> p a b m", a=2, b=9)
                nc.gpsimd.dma_start(out=wh, in_=src_ap)
                return wh

            def smalls_layer(xs, ys, st, pspool, xps, wt_t, pk_t, levels):
                held = {}
                for l in levels:
                    H, W = LEVELS[l]
                    tiles = _tiles_for(l)
                    y = ys.tile([128, 2, H, W], f32, tag=f"y{l}")
                    stats_t = st.tile([128, 2, len(tiles), 6], f32, tag=f"stats{l}")
                    for ti, (r0, R) in enumerate(tiles):
                        N = R * W
                        for mh in range(2):
                            ps = pspool.tile([128, 512], f32, tag=f"ps{mh}")
                            conv_tile(
                                ps[:, :N].rearrange("p (r w) -> p r w", r=R),
                                wt_t, xps[l], l, ("tower", mh), r0, R,
                            )
                            nc.scalar.activation(
                                y[:, mh, r0 : r0 + R, :],
                                ps[:, :N].rearrange("p (r w) -> p r w", r=R),
                                AF.Copy,
                            )
                            nc.vector.bn_stats(
                                out=stats_t[:, mh, ti, :],
                                in_=y[:, mh, r0 : r0 + R, :].rearrange(
                                    "p r w -> p (r w)"
                                ),
                            )
                    held[l] = (y, stats_t, tiles)
                for l in levels:
                    H, W = LEVELS[l]
                    y, stats_t, tiles = held[l]
                    ab = gn_finalize(st, pspool, stats_t, pk_t, tiles, W)
                    xp_new = xs.tile([128, 2, H + 2, W + 2], f32r, tag=f"xp{l}")
                    memset_borders(xp_new, l)
                    for h in range(2):
                        nc.scalar.activation(
                            xp_new[:, h, 1 : H + 1, 1 : W + 1],
                            y[:, h],
                            AF.Relu,
                            scale=ab[:, h : h + 1],
                            bias=ab[:, 2 + h : 3 + h],
                        )
                    xps[l] = xp_new

            if "A" in _phases:
                with tc.tile_pool(name="wp", bufs=2) as wp, \
                     tc.tile_pool(name="xs", bufs=2) as xs, \
                     tc.tile_pool(name="ys", bufs=1) as ys, \
                     tc.tile_pool(name="st", bufs=2) as st, \
                     tc.tile_pool(name="hp", bufs=4) as hp, \
                     tc.tile_pool(name="ps", bufs=2, space="PSUM") as pspool:
                    xps = load_levels(xs, A_LEVELS)
                    pre_w = None
                    for t in ("c", "b"):
                        for L in range(NUM_CONVS):
                            if L == 0 and pre_w is not None:
                                wt_t, pk_t = pre_w
                            else:
                                wt_t, pk_t = load_w(wp, st, t, L)
                            smalls_layer(xs, ys, st, pspool, xps, wt_t, pk_t, A_LEVELS)
                        if t == "c":
                            # prefetch bbox inputs + first weights during heads
                            xps_b = load_levels(xs, A_LEVELS)
                            pre_w = load_w(wp, st, "b", 0)
                        wh = load_head_w(wp, t)
                        for l in A_LEVELS:
                            if t == "c":
                                head_convs(xps[l], l, pspool, hp, wh, None)
                            else:
                                head_convs(xps[l], l, pspool, hp, None, wh)
                        if t == "c":
                            xps = xps_b

            # =====================================================
            # Phase B: level 0, streamed through DRAM scratch.
            # Layer 0 runs BOTH towers off the single initial load; the
            # bbox path restarts later from its parked DRAM conv output.
            # =====================================================
            H, W = LEVELS[0]
            tiles0 = _tiles_for(0)
            if "B" in _phases:
                wp = wp0
                with tc.tile_pool(name="x0", bufs=1) as x0p, \
                     tc.tile_pool(name="st0", bufs=2) as st, \
                     tc.tile_pool(name="stg", bufs=3) as stg, \
                     tc.tile_pool(name="rfl", bufs=2, space="SBUF") as rfl, \
                     tc.tile_pool(name="hp0", bufs=2) as hp, \
                     tc.tile_pool(name="xs34", bufs=2) as xs34, \
                     tc.tile_pool(name="ys34", bufs=1) as ys34, \
                     tc.tile_pool(name="dr", bufs=2, space="DRAM") as drp, \
                     tc.tile_pool(name="ps0", bufs=2, space="PSUM") as pspool:
                    # weights first: the first conv only needs wc0 + a few
                    # input rows, so don't queue 13MB of image ahead of it
                    wc0, pc0 = load_w(wp, st, "c", 0)
                    xp = x0p.tile([128, 2, H + 2, W + 2], f32r, tag="xp0")
                    memset_borders(xp, 0)
                    # chunked load: row-range deps let layer-0 convs start
                    # as soon as their input rows land
                    for r in range(0, H, F0_REFILL_ROWS):
                        RR = min(F0_REFILL_ROWS, H - r)
                        for kh in range(2):
                            nc.gpsimd.dma_start(
                                out=xp[:, kh, r + 1 : r + RR + 1, 1 : W + 1],
                                in_=feats[0][128 * kh : 128 * (kh + 1), r : r + RR, :],
                            )

                    def f0_conv_layer(wt_t, yraw, stats_t):
                        for ti, (r0, R) in enumerate(tiles0):
                            N = R * W
                            sg = stg.tile([128, 2, 512], f32, tag="sg")
                            for mh in range(2):
                                ps = pspool.tile([128, 512], f32, tag=f"ps{mh}")
                                conv_tile(
                                    ps[:, :N].rearrange("p (r w) -> p r w", r=R),
                                    wt_t, xp, 0, ("tower", mh), r0, R,
                                )
                                nc.scalar.activation(sg[:, mh, :N], ps[:, :N], AF.Copy)
                                nc.vector.bn_stats(
                                    out=stats_t[:, mh, ti, :], in_=sg[:, mh, :N]
                                )
                            nc.sync.dma_start(
                                out=yraw[:, :, r0 * W : r0 * W + N], in_=sg[:, :, :N]
                            )

                    def f0_refill_chunk(yraw, ab, r, RR=None):
                        RR = min(RR or F0_REFILL_ROWS, H - r)
                        rt = rfl.tile([128, 2, F0_REFILL_ROWS, W], f32, tag="rt")
                        nc.sync.dma_start(
                            out=rt[:, :, :RR, :].rearrange("p a r w -> p a (r w)"),
                            in_=yraw[:, :, r * W : (r + RR) * W],
                        )
                        for h in range(2):
                            nc.scalar.activation(
                                xp[:, h, r + 1 : r + RR + 1, 1 : W + 1],
                                rt[:, h, :RR, :],
                                AF.Relu,
                                scale=ab[:, h : h + 1],
                                bias=ab[:, 2 + h : 3 + h],
                            )

                    def f0_refill(yraw, ab):
                        # small leading chunk: the next layer's first conv
                        # tile only needs ~5 rows, so publish them early
                        f0_refill_chunk(yraw, ab, 0, 4)
                        r = 4
                        while r < H:
                            f0_refill_chunk(yraw, ab, r)
                            r += F0_REFILL_ROWS

                    # levels 3/4 ride along as PE filler between f0 layers
                    xp34h = [load_34(xs34)]

                    # layer 0, both towers, off the pristine input
                    yraw_c = drp.tile([128, 2, H * W], f32, tag="yrc")
                    stats_c = st.tile([128, 2, len(tiles0), 6], f32, tag="stats0")
                    f0_conv_layer(wc0, yraw_c, stats_c)
                    wb0, pb0 = load_w(wp, st, "b", 0)
                    pb0b = st.tile([128, 2, 3], f32, tag="pkb")
                    nc.vector.tensor_copy(pb0b, pb0)
                    yraw_b = drp.tile([128, 2, H * W], f32, tag="yrb")
                    stats_b = st.tile([128, 2, len(tiles0), 6], f32, tag="stats0b")
                    f0_conv_layer(wb0, yraw_b, stats_b)
                    smalls34_layer(xs34, ys34, st, pspool, xp34h, wc0, pc0)
                    ab_b = gn_finalize(st, pspool, stats_b, pb0b, tiles0, W, "b")
                    ab_c = gn_finalize(st, pspool, stats_c, pc0, tiles0, W, "0")
                    f0_refill(yraw_c, ab_c)

                    for t in ("c", "b"):
                        lo = 1 if t == "c" else 0
                        for L in range(lo, NUM_CONVS):
                            wt_t, pk_t = load_w(wp, st, t, L)
                            if L >= 1:
                                yraw = drp.tile([128, 2, H * W], f32, tag="yrc")
                                stats_t = st.tile(
                                    [128, 2, len(tiles0), 6], f32, tag="stats0"
                                )
                                f0_conv_layer(wt_t, yraw, stats_t)
                            smalls34_layer(
                                xs34, ys34, st, pspool, xp34h, wt_t, pk_t
                            )
                            if L >= 1:
                                ab = gn_finalize(
                                    st, pspool, stats_t, pk_t, tiles0, W, "0"
                                )
                                f0_refill(yraw, ab)
                        wh = load_head_w(wp, t)
                        if t == "c":
                            # bbox path restarts from the parked layer-0 output;
                            # its refill chunks interleave with cls head tiles
                            # right after each chunk's last WAR reader
                            inter = {}
                            nt0 = len(tiles0)
                            for k, r in enumerate(range(0, H, F0_REFILL_ROWS)):
                                last_reader = min(
                                    nt0 - 1, (r + F0_REFILL_ROWS) // ROWS[0]
                                )
                                inter.setdefault(last_reader, []).append(r)

                            def mk(rs):
                                return lambda: [
                                    f0_refill_chunk(yraw_b, ab_b, r) for r in rs
                                ]

                            inter = {ti: mk(rs) for ti, rs in inter.items()}
                            head_convs(xp, 0, pspool, hp, wh, None, interleave=inter)
                            heads34(pspool, hp, xp34h, wh, None)
                            # fresh l3/l4 inputs for the bbox tower
                            xp34h = [load_34(xs34)]
                        else:
                            head_convs(xp, 0, pspool, hp, None, wh)
                            heads34(pspool, hp, xp34h, None, wh)

            wp0_pool.__exit__(None, None, None)

    nc.compile()
    return nc


# ---------------------------------------------------------------- entry
_CACHE = {}


def kernel(f0, f1, f2, f3, f4, params):
    from concourse.bass_utils import run_bass_kernel_spmd

    feats = [_np(f0), _np(f1), _np(f2), _np(f3), _np(f4)]
    scales = _np(params["scales"])

    wmap = {}
    for t, key in (("c", "cls_tower"), ("b", "bbox_tower")):
        for L, (W, b, g, be) in enumerate(params[key]):
            wmap[f"w{t}{L}"] = _prep_tower_w(W)
            pk = np.stack([_np(g), _np(be), _np(b)], axis=1)  # [256, 3]
            wmap[f"p{t}{L}"] = np.ascontiguousarray(
                pk.reshape(2, 128, 3).transpose(1, 0, 2).reshape(128, 6)
            )
    wmap["wch"], _ = _prep_head_w([params["cls_W"]])
    wmap["wbh"], _ = _prep_head_w([params["box_W"], params["ctr_W"]])
    wmap["hbc"] = _np(params["cls_b"]).reshape(16, 1)
    hbb = np.zeros((4, 5), np.float32)
    for l in range(5):
        hbb[:, l] = _np(params["box_b"]) * float(scales[l]) * float(STRIDES[l])
    wmap["hbb"] = hbb
    ind16 = np.zeros((128, 16), np.float32)
    for g in range(16):
        ind16[8 * g : 8 * (g + 1), g] = 0.125
    wmap["ind16"] = ind16
    bc16 = np.zeros((16, 128), np.float32)
    for g in range(16):
        bc16[g, 8 * g : 8 * (g + 1)] = 1.0
    wmap["bc16"] = bc16

    key = scales.tobytes()
    if key not in _CACHE:
        _CACHE[key] = build_program(scales)
    nc = _CACHE[key]

    in_maps = []
    for b in range(NCORES):
        m = {f"f{l}": feats[l][b] for l in range(5)}
        m.update(wmap)
        in_maps.append(m)

    # the axon/PJRT execute occasionally faults transiently
    # (NRT_EXEC_UNIT_UNRECOVERABLE); retry a couple of times
    import time

    last = None
    for attempt in range(3):
        try:
            res = run_bass_kernel_spmd(nc, in_maps, core_ids=list(range(NCORES)))
            break
        except Exception as e:  # noqa: BLE001
            last = e
            if attempt == 2:
                raise
            time.sleep(5.0)
    out = np.stack([res.results[b]["out"] for b in range(NCORES)], axis=0)
    return out, _locations()


# revision 53
# speedup vs baseline: 1.0091x; 1.0064x over previous
"""FCOS head (nms_detection) Trainium2 Bass kernel.

Strategy: data-parallel over batch across 8 NeuronCores (1 image/core,
weights replicated).  Per core, each 3x3 SAME conv is computed as 18
accumulating float32r matmuls (9 taps x 2 input-channel halves) per
<=512-pixel output tile, reading from a zero-padded SBUF activation
image.  GroupNorm statistics come from bn_stats/bn_aggr per channel,
then two tiny exact-fp32 matmuls do the cross-partition group reduce
and broadcast; normalize+ReLU is a single fused scalar-engine
activation (Relu(A*x+B)) per channel-half.  Level 0 (100x128) streams
conv outputs through a DRAM scratch buffer (its x and y don't both fit
in SBUF); levels 1-4 stay SBUF-resident.  Head convs (cls / box+ctr)
use the same tap-matmul scheme with M=16 / M=5.
"""

import math

import numpy as np

# ---------------------------------------------------------------- constants
IN_CH = 256
NUM_CLASSES = 16
NUM_CONVS = 4
STRIDES = (8, 16, 32, 64, 128)
GN_EPS = 1e-5
NCORES = 8

# (H, W) per level
LEVELS = [(100, 128), (50, 64), (25, 32), (13, 16), (7, 8)]
# output-row chunk per level (rows*W <= 512).  Chunk sizes may be unequal;
# gn stats aggregate per size-group and combine with host-known weights.
ROWS = [4, 8, 16, 13, 7]
OFFS = [0, 12800, 16000, 16800, 17008]
TOTAL_LOCS = 17064

F0_REFILL_ROWS = 8  # rows per refill chunk for level 0


def _np(x):
    return np.ascontiguousarray(np.asarray(x), dtype=np.float32)


def _tiles_for(level):
    H, _ = LEVELS[level]
    R = ROWS[level]
    out = []
    r = 0
    while r < H:
        out.append((r, min(R, H - r)))
        r += R
    return out


def _prep_tower_w(W):
    # W [O=256, I=256, 3, 3] -> [p=128, kh=2, tap=9, mh=2, m=128] flattened
    W = _np(W).reshape(2, 128, 2, 128, 3, 3)  # [mh, m, kh, p, dy, dx]
    wt = np.transpose(W, (3, 2, 4, 5, 0, 1))  # [p, kh, dy, dx, mh, m]
    return np.ascontiguousarray(wt.reshape(128, 2 * 9 * 2 * 128))


def _prep_head_w(Ws):
    # list of [o_i, 256, 3, 3] stacked on o -> [p, kh, tap, m_total]
    W = np.concatenate([_np(w) for w in Ws], axis=0)  # [M, 256, 3, 3]
    M = W.shape[0]
    W = W.reshape(M, 2, 128, 3, 3)  # [m, kh, p, dy, dx]
    wt = np.transpose(W, (2, 1, 3, 4, 0))  # [p, kh, dy, dx, m]
    return np.ascontiguousarray(wt.reshape(128, 2 * 9 * M)), M


def _locations():
    locs = []
    for l, (h, w) in enumerate(LEVELS):
        s = STRIDES[l]
        sx = np.arange(0, w * s, s, dtype=np.float32)
        sy = np.arange(0, h * s, s, dtype=np.float32)
        yy, xx = np.meshgrid(sy, sx, indexing="ij")
        locs.append(np.stack([xx.reshape(-1), yy.reshape(-1)], axis=1) + s // 2)
    return np.concatenate(locs, axis=0).astype(np.float32)


# ---------------------------------------------------------------- program
def build_program(scales):
    import concourse.bacc as bacc
    import concourse.mybir as mybir
    import concourse.tile as tile

    f32 = mybir.dt.float32
    f32r = mybir.dt.float32r
    AF = mybir.ActivationFunctionType

    nc = bacc.Bacc(trn_type="TRN2", num_swdge_queues=4)

    feats = [
        nc.declare_dram_parameter(f"f{l}", [IN_CH, H, W], f32, isOutput=False)
        for l, (H, W) in enumerate(LEVELS)
    ]
    wts = {}
    pks = {}
    for t in ("c", "b"):
        for L in range(NUM_CONVS):
            wts[(t, L)] = nc.declare_dram_parameter(
                f"w{t}{L}", [128, 4608], f32, isOutput=False
            )
            pks[(t, L)] = nc.declare_dram_parameter(
                f"p{t}{L}", [128, 6], f32, isOutput=False
            )
    wch = nc.declare_dram_parameter("wch", [128, 288], f32, isOutput=False)
    wbh = nc.declare_dram_parameter("wbh", [128, 90], f32, isOutput=False)
    hbc = nc.declare_dram_parameter("hbc", [16, 1], f32, isOutput=False)
    hbb = nc.declare_dram_parameter("hbb", [4, 5], f32, isOutput=False)
    gavg_d = nc.declare_dram_parameter("gavg", [128, 128], f32, isOutput=False)
    out = nc.declare_dram_parameter("out", [21, TOTAL_LOCS], f32, isOutput=True)

    with tile.TileContext(nc) as tc:
        # ---- constants that live for the whole kernel
        with tc.tile_pool(name="konst", bufs=1) as kp:
            gavg = kp.tile([128, 128], f32)
            nc.sync.dma_start(out=gavg, in_=gavg_d[:, :])
            hbc_t = kp.tile([16, 1], f32)
            nc.sync.dma_start(out=hbc_t, in_=hbc[:, :])
            hbb_t = kp.tile([4, 5], f32)
            nc.sync.dma_start(out=hbb_t, in_=hbb[:, :])
            epst = kp.tile([128, 1], f32)
            nc.vector.memset(epst, GN_EPS)

            def conv_tile(ps, wt_t, xp, level, kind, r0, R, width=None):
                """Accumulate 3x3 conv for output rows [r0, r0+R) into psum.

                kind: ("tower", mh) or ("head", M) selecting lhsT slice.
                wt_t viewed as [128, 2, 9, X]."""
                W = width if width is not None else LEVELS[level][1]
                first = True
                for kh in range(2):
                    for tap in range(9):
                        dy, dx = tap // 3, tap % 3
                        rhs = xp[:, kh, r0 + dy : r0 + dy + R, dx : dx + W]
                        if kind[0] == "tower":
                            lhsT = wt_t[:, kh, tap, kind[1], :]
                        else:
                            lhsT = wt_t[:, kh, tap, : kind[1]]
                        nc.tensor.matmul(
                            ps,
                            lhsT,
                            rhs,
                            start=first,
                            stop=(kh == 1 and tap == 8),
                        )
                        first = False

            def gn_finalize(spool, pspool, stats_t, pk_t, tile_list, W, tagsfx=""):
                """Per-channel chunk stats -> per-channel affine (A,B) [128,4].

                Chunks may have unequal pixel counts: bn_aggr is unweighted, so
                aggregate per size-group and combine with host-known weights."""
                Ns = [R * W for (_, R) in tile_list]
                groups = []  # (ti_start, ti_end, chunk_px)
                for ti, n in enumerate(Ns):
                    if groups and groups[-1][2] == n:
                        groups[-1][1] = ti + 1
                    else:
                        groups.append([ti, ti + 1, n])
                total = float(sum(Ns))
                # me cols: mean0 e2_0 mean1 e2_1 (bias-free mean and E[x^2])
                me = spool.tile([128, 4], f32, tag="me" + tagsfx)
                sq = spool.tile([128, 1], f32, tag="sq" + tagsfx)
                mv = spool.tile([128, 2], f32, tag="mv" + tagsfx)
                acc = spool.tile([128, 2], f32, tag="macc" + tagsfx)
                for h in range(2):
                    if len(groups) == 1:
                        nc.vector.bn_aggr(out=mv, in_=stats_t[:, h])
                        nc.vector.tensor_copy(me[:, 2 * h : 2 * h + 1], mv[:, 0:1])
                        nc.vector.tensor_mul(sq, mv[:, 0:1], mv[:, 0:1])
                        nc.vector.tensor_add(
                            me[:, 2 * h + 1 : 2 * h + 2], mv[:, 1:2], sq
                        )
                    else:
                        first = True
                        for (s, e, n) in groups:
                            w_g = (e - s) * n / total
                            nc.vector.bn_aggr(out=mv, in_=stats_t[:, h, s:e])
                            # e2_g = var + mean^2 ; acc += w_g * [mean, e2_g]
                            nc.vector.tensor_mul(sq, mv[:, 0:1], mv[:, 0:1])
                            nc.vector.tensor_add(mv[:, 1:2], mv[:, 1:2], sq)
                            nc.vector.tensor_scalar_mul(mv, mv, w_g)
                            if first:
                                nc.vector.tensor_copy(acc, mv)
                                first = False
                            else:
                                nc.vector.tensor_add(acc, acc, mv)
                        nc.vector.tensor_copy(me[:, 2 * h : 2 * h + 1], acc[:, 0:1])
                        nc.vector.tensor_copy(
                            me[:, 2 * h + 1 : 2 * h + 2], acc[:, 1:2]
                        )
                # mbuf cols: m0 s0 m1 s1 ; m = mean + b, s = e2 + b*(mean + m)
                mbuf = spool.tile([128, 4], f32, tag="mbuf" + tagsfx)
                for h in range(2):
                    nc.vector.tensor_add(
                        mbuf[:, 2 * h : 2 * h + 1], me[:, 2 * h : 2 * h + 1],
                        pk_t[:, h, 2:3],
                    )
                    nc.vector.tensor_add(
                        sq, me[:, 2 * h : 2 * h + 1], mbuf[:, 2 * h : 2 * h + 1]
                    )
                    nc.vector.tensor_mul(sq, sq, pk_t[:, h, 2:3])
                    nc.vector.tensor_add(
                        mbuf[:, 2 * h + 1 : 2 * h + 2],
                        me[:, 2 * h + 1 : 2 * h + 2], sq,
                    )
                # fused block-diagonal group-average+broadcast (exact: the
                # off-group K entries are hard zeros)
                bps = pspool.tile([128, 4], f32, tag="pstat")
                nc.tensor.matmul(bps, gavg, mbuf, start=True, stop=True)
                bsb = spool.tile([128, 4], f32, tag="bsb" + tagsfx)
                nc.scalar.activation(bsb, bps, AF.Copy)
                # var = E2 - mu^2 ; rstd = 1/sqrt(var+eps)
                ab = spool.tile([128, 4], f32, tag="ab" + tagsfx)  # A0 A1 B0 B1
                var = spool.tile([128, 2], f32, tag="var" + tagsfx)
                for h in range(2):
                    nc.vector.tensor_mul(sq, bsb[:, 2 * h : 2 * h + 1], bsb[:, 2 * h : 2 * h + 1])
                    nc.vector.tensor_sub(var[:, h : h + 1], bsb[:, 2 * h + 1 : 2 * h + 2], sq)
                    nc.scalar.activation(
                        var[:, h : h + 1], var[:, h : h + 1], AF.Sqrt, bias=epst[:, 0:1]
                    )
                    nc.vector.reciprocal(var[:, h : h + 1], var[:, h : h + 1])
                    nc.vector.tensor_mul(
                        ab[:, h : h + 1], var[:, h : h + 1], pk_t[:, h, 0:1]
                    )
                    # B = beta + A*(bias - mu)
                    nc.vector.tensor_sub(sq, pk_t[:, h, 2:3], bsb[:, 2 * h : 2 * h + 1])
                    nc.vector.tensor_mul(sq, ab[:, h : h + 1], sq)
                    nc.vector.tensor_add(ab[:, 2 + h : 3 + h], pk_t[:, h, 1:2], sq)
                return ab

            def memset_borders(xp, level):
                H, W = LEVELS[level]
                for h in range(2):
                    nc.vector.memset(xp[:, h, 0:1, :].bitcast(f32), 0.0)
                    nc.vector.memset(xp[:, h, H + 1 : H + 2, :].bitcast(f32), 0.0)
                    nc.vector.memset(xp[:, h, 1 : H + 1, 0:1].bitcast(f32), 0.0)
                    nc.vector.memset(
                        xp[:, h, 1 : H + 1, W + 1 : W + 2].bitcast(f32), 0.0
                    )

            def head_convs(xp, level, pspool, hpool, wch_t, wbh_t, interleave=None):
                """cls head if wch_t else box+ctr head, from padded tower out.

                interleave: optional {tile_idx: callable} of filler work to
                emit between head tiles (keeps engine FIFOs pipelined)."""
                H, W = LEVELS[level]
                off = OFFS[level]
                for ti, (r0, R) in enumerate(_tiles_for(level)):
                    N = R * W
                    cols = slice(off + r0 * W, off + r0 * W + N)
                    if wch_t is not None:
                        ps = pspool.tile([128, 512], f32, tag=f"ps{ti % 2}")
                        conv_tile(
                            ps[:16, :N].rearrange("p (r w) -> p r w", r=R),
                            wch_t, xp, level, ("head", 16), r0, R,
                        )
                        hs = hpool.tile([16, 512], f32, tag="hscls")
                        nc.vector.tensor_scalar_add(hs[:, :N], ps[:16, :N], hbc_t[:, 0:1])
                        nc.sync.dma_start(out=out[0:16, cols], in_=hs[:, :N])
                    else:
                        s = float(scales[level]) * float(STRIDES[level])
                        ps = pspool.tile([128, 512], f32, tag=f"ps{ti % 2}")
                        conv_tile(
                            ps[:5, :N].rearrange("p (r w) -> p r w", r=R),
                            wbh_t, xp, level, ("head", 5), r0, R,
                        )
                        hs = hpool.tile([5, 512], f32, tag="hsbc")
                        hr = hpool.tile([5, 512], f32, tag="hsraw")
                        nc.vector.tensor_copy(hr[:, :N], ps[:5, :N])
                        # box rows: relu(s*t*x + b*s*t) = s*t*relu(x + b), s,t>0
                        nc.vector.tensor_scalar(
                            hs[:4, :N], ps[:4, :N],
                            s, hbb_t[:, level : level + 1],
                            op0=mybir.AluOpType.mult, op1=mybir.AluOpType.add,
                        )
                        nc.vector.tensor_scalar_max(hs[:4, :N], hs[:4, :N], 0.0)
                        nc.sync.dma_start(out=out[16:20, cols], in_=hs[:4, :N])
                        nc.sync.dma_start(out=out[20:21, cols], in_=hr[4:5, :N])
                    if interleave and ti in interleave:
                        interleave[ti]()

            # ---- combined level 3+4 image: l3 (13x16) rows 0..12, two
            # zero separator rows, l4 (7x8) rows 15..21 in cols 0..7.
            # One 352-px conv tile covers both levels (junk rows 13/14 unused).
            HC, WC = 22, 16

            def load_34(xs):
                xp = xs.tile([128, 2, HC + 2, WC + 2], f32r, tag="xp34")
                for h in range(2):
                    nc.vector.memset(xp[:, h, 0:1, :].bitcast(f32), 0.0)
                    nc.vector.memset(xp[:, h, 14:16, :].bitcast(f32), 0.0)
                    nc.vector.memset(xp[:, h, 23:24, :].bitcast(f32), 0.0)
                    nc.vector.memset(xp[:, h, 1 : HC + 1, 0:1].bitcast(f32), 0.0)
                    nc.vector.memset(
                        xp[:, h, 1 : HC + 1, WC + 1 : WC + 2].bitcast(f32), 0.0
                    )
                    nc.vector.memset(xp[:, h, 16:23, 9:17].bitcast(f32), 0.0)
                for kh in range(2):
                    nc.gpsimd.dma_start(
                        out=xp[:, kh, 1:14, 1:17],
                        in_=feats[3][128 * kh : 128 * (kh + 1), :, :],
                    )
                    nc.gpsimd.dma_start(
                        out=xp[:, kh, 16:23, 1:9],
                        in_=feats[4][128 * kh : 128 * (kh + 1), :, :],
                    )
                return xp

            def smalls34_layer(xs, ys, st, pspool, holder, wt_t, pk_t):
                xp = holder[0]
                y = ys.tile([128, 2, HC, WC], f32, tag="y34")
                st3 = st.tile([128, 2, 1, 6], f32, tag="stats3")
                st4 = st.tile([128, 2, 7, 6], f32, tag="stats4")
                for mh in range(2):
                    ps = pspool.tile([128, 512], f32, tag=f"ps{mh}")
                    conv_tile(
                        ps[:, : HC * WC].rearrange("p (r w) -> p r w", r=HC),
                        wt_t, xp, None, ("tower", mh), 0, HC, width=WC,
                    )
                    nc.scalar.activation(
                        y[:, mh],
                        ps[:, : HC * WC].rearrange("p (r w) -> p r w", r=HC),
                        AF.Copy,
                    )
                    nc.vector.bn_stats(
                        out=st3[:, mh, 0, :],
                        in_=y[:, mh, 0:13, :].rearrange("p r w -> p (r w)"),
                    )
                    for i in range(7):
                        nc.vector.bn_stats(
                            out=st4[:, mh, i, :], in_=y[:, mh, 15 + i, 0:8]
                        )
                ab3 = gn_finalize(st, pspool, st3, pk_t, [(0, 13)], 16, "3")
                ab4 = gn_finalize(st, pspool, st4, pk_t, [(i, 1) for i in range(7)], 8, "4")
                xp_new = xs.tile([128, 2, HC + 2, WC + 2], f32r, tag="xp34")
                for h in range(2):
                    nc.vector.memset(xp_new[:, h, 0:1, :].bitcast(f32), 0.0)
                    nc.vector.memset(xp_new[:, h, 14:16, :].bitcast(f32), 0.0)
                    nc.vector.memset(xp_new[:, h, 23:24, :].bitcast(f32), 0.0)
                    nc.vector.memset(xp_new[:, h, 1 : HC + 1, 0:1].bitcast(f32), 0.0)
                    nc.vector.memset(
                        xp_new[:, h, 1 : HC + 1, WC + 1 : WC + 2].bitcast(f32), 0.0
                    )
                    nc.vector.memset(xp_new[:, h, 16:23, 9:17].bitcast(f32), 0.0)
                    nc.scalar.activation(
                        xp_new[:, h, 1:14, 1:17],
                        y[:, h, 0:13, :],
                        AF.Relu,
                        scale=ab3[:, h : h + 1],
                        bias=ab3[:, 2 + h : 3 + h],
                    )
                    nc.scalar.activation(
                        xp_new[:, h, 16:23, 1:9],
                        y[:, h, 15:22, 0:8],
                        AF.Relu,
                        scale=ab4[:, h : h + 1],
                        bias=ab4[:, 2 + h : 3 + h],
                    )
                holder[0] = xp_new

            def heads34(pspool, hpool, holder, wch_t, wbh_t):
                xp = holder[0]
                is_cls = wch_t is not None
                M = 16 if is_cls else 5
                wt_t = wch_t if is_cls else wbh_t
                ps = pspool.tile([128, 512], f32, tag="ps0")
                conv_tile(
                    ps[:M, : HC * WC].rearrange("p (r w) -> p r w", r=HC),
                    wt_t, xp, None, ("head", M), 0, HC, width=WC,
                )
                psv = ps[:, : HC * WC].rearrange("p (r w) -> p r w", r=HC)
                if is_cls:
                    hs = hpool.tile([16, HC, WC], f32, tag="hscls")
                    nc.vector.tensor_scalar_add(hs, psv[:16], hbc_t[:, 0:1])
                    nc.sync.dma_start(
                        out=out[0:16, OFFS[3] : OFFS[3] + 208],
                        in_=hs[:, 0:13, :].rearrange("p r w -> p (r w)"),
                    )
                    nc.sync.dma_start(
                        out=out[0:16, OFFS[4] : OFFS[4] + 56].rearrange(
                            "p (r w) -> p r w", r=7
                        ),
                        in_=hs[:, 15:22, 0:8],
                    )
                else:
                    hs = hpool.tile([5, HC, WC], f32, tag="hsbc")
                    hr = hpool.tile([5, HC, WC], f32, tag="hsraw")
                    nc.vector.tensor_copy(hr, psv[:5])
                    for lvl, rows, colw in ((3, slice(0, 13), 16), (4, slice(15, 22), 8)):
                        s = float(scales[lvl]) * float(STRIDES[lvl])
                        nc.vector.tensor_scalar(
                            hs[:4, rows, :colw], psv[:4, rows, :colw],
                            s, hbb_t[:, lvl : lvl + 1],
                            op0=mybir.AluOpType.mult, op1=mybir.AluOpType.add,
                        )
                        nc.vector.tensor_scalar_max(
                            hs[:4, rows, :colw], hs[:4, rows, :colw], 0.0
                        )
                    nc.sync.dma_start(
                        out=out[16:20, OFFS[3] : OFFS[3] + 208],
                        in_=hs[:4, 0:13, :].rearrange("p r w -> p (r w)"),
                    )
                    nc.sync.dma_start(
                        out=out[20:21, OFFS[3] : OFFS[3] + 208],
                        in_=hr[4:5, 0:13, :].rearrange("p r w -> p (r w)"),
                    )
                    nc.sync.dma_start(
                        out=out[16:20, OFFS[4] : OFFS[4] + 56].rearrange(
                            "p (r w) -> p r w", r=7
                        ),
                        in_=hs[:4, 15:22, 0:8],
                    )
                    nc.sync.dma_start(
                        out=out[20:21, OFFS[4] : OFFS[4] + 56].rearrange(
                            "p (r w) -> p r w", r=7
                        ),
                        in_=hr[4:5, 15:22, 0:8],
                    )

            # =====================================================
            # Phase A: levels 1-4, tower-major (weights loaded once/layer)
            # =====================================================
            import os as _os
            _phases = _os.environ.get("KPHASES", "AB")
            A_LEVELS = [1, 2]   # levels 3/4 ride along in phase B as filler
            B_LEVELS = [3, 4]
            # phase-B weight pool opened early: its SBUF never overlaps
            # phase A's pools, so B's first weight DMA can run during A
            wp0_pool = tc.tile_pool(name="wp0", bufs=2)
            wp0 = wp0_pool.__enter__()

            def load_levels(xs, levels):
                xps = {}
                for l in levels:
                    H, W = LEVELS[l]
                    xp = xs.tile([128, 2, H + 2, W + 2], f32r, tag=f"xp{l}")
                    memset_borders(xp, l)
                    for kh in range(2):
                        nc.gpsimd.dma_start(
                            out=xp[:, kh, 1 : H + 1, 1 : W + 1],
                            in_=feats[l][128 * kh : 128 * (kh + 1), :, :],
                        )
                    xps[l] = xp
                return xps

            def load_w(wp, st, t, L):
                wt_t = wp.tile([128, 2, 9, 2, 128], f32r, tag="w")
                # split by input-channel half: conv_tile consumes kh=0 taps
                # first, so the first matmuls start after half the DMA
                src = wts[(t, L)][:, :].rearrange(
                    "p (a b c d) -> p a b c d", a=2, b=9, c=2
                )
                for kh in range(2):
                    nc.gpsimd.dma_start(
                        out=wt_t[:, kh : kh + 1], in_=src[:, kh : kh + 1]
                    )
                pk_t = st.tile([128, 2, 3], f32, tag="pk")
                nc.sync.dma_start(
                    out=pk_t, in_=pks[(t, L)][:, :].rearrange("p (a b) -> p a b", a=2)
                )
                return wt_t, pk_t

            def load_head_w(wp, t):
                if t == "c":
                    wh = wp.tile([128, 2, 9, 16], f32r, tag="wh")
                    src_ap = wch[:, :].rearrange("p (a b m) -> p a b m", a=2, b=9)
                else:
                    wh = wp.tile([128, 2, 9, 5], f32r, tag="wh")
                    src_ap = wbh[:, :].rearrange("p (a b m) -> p a b m", a=2, b=9)
                nc.gpsimd.dma_start(out=wh, in_=src_ap)
                return wh

            def smalls_layer(xs, ys, st, pspool, xps, wt_t, pk_t, levels):
                held = {}
                for l in levels:
                    H, W = LEVELS[l]
                    tiles = _tiles_for(l)
                    y = ys.tile([128, 2, H, W], f32, tag=f"y{l}")
                    stats_t = st.tile([128, 2, len(tiles), 6], f32, tag=f"stats{l}")
                    for ti, (r0, R) in enumerate(tiles):
                        N = R * W
                        for mh in range(2):
                            ps = pspool.tile([128, 512], f32, tag=f"ps{mh}")
                            conv_tile(
                                ps[:, :N].rearrange("p (r w) -> p r w", r=R),
                                wt_t, xps[l], l, ("tower", mh), r0, R,
                            )
                            nc.scalar.activation(
                                y[:, mh, r0 : r0 + R, :],
                                ps[:, :N].rearrange("p (r w) -> p r w", r=R),
                                AF.Copy,
                            )
                            nc.vector.bn_stats(
                                out=stats_t[:, mh, ti, :],
                                in_=y[:, mh, r0 : r0 + R, :].rearrange(
                                    "p r w -> p (r w)"
                                ),
                            )
                    held[l] = (y, stats_t, tiles)
                for l in levels:
                    H, W = LEVELS[l]
                    y, stats_t, tiles = held[l]
                    ab = gn_finalize(st, pspool, stats_t, pk_t, tiles, W)
                    xp_new = xs.tile([128, 2, H + 2, W + 2], f32r, tag=f"xp{l}")
                    memset_borders(xp_new, l)
                    for h in range(2):
                        nc.scalar.activation(
                            xp_new[:, h, 1 : H + 1, 1 : W + 1],
                            y[:, h],
                            AF.Relu,
                            scale=ab[:, h : h + 1],
                            bias=ab[:, 2 + h : 3 + h],
                        )
                    xps[l] = xp_new

            if "A" in _phases:
                with tc.tile_pool(name="wp", bufs=2) as wp, \
                     tc.tile_pool(name="xs", bufs=2) as xs, \
                     tc.tile_pool(name="ys", bufs=1) as ys, \
                     tc.tile_pool(name="st", bufs=2) as st, \
                     tc.tile_pool(name="hp", bufs=4) as hp, \
                     tc.tile_pool(name="ps", bufs=2, space="PSUM") as pspool:
                    xps = load_levels(xs, A_LEVELS)
                    pre_w = None
                    for t in ("c", "b"):
                        for L in range(NUM_CONVS):
                            if L == 0 and pre_w is not None:
                                wt_t, pk_t = pre_w
                            else:
                                wt_t, pk_t = load_w(wp, st, t, L)
                            smalls_layer(xs, ys, st, pspool, xps, wt_t, pk_t, A_LEVELS)
                        if t == "c":
                            # prefetch bbox inputs + first weights during heads
                            xps_b = load_levels(xs, A_LEVELS)
                            pre_w = load_w(wp, st, "b", 0)
                        wh = load_head_w(wp, t)
                        for l in A_LEVELS:
                            if t == "c":
                                head_convs(xps[l], l, pspool, hp, wh, None)
                            else:
                                head_convs(xps[l], l, pspool, hp, None, wh)
                        if t == "c":
                            xps = xps_b

            # =====================================================
            # Phase B: level 0, streamed through DRAM scratch.
            # Layer 0 runs BOTH towers off the single initial load; the
            # bbox path restarts later from its parked DRAM conv output.
            # =====================================================
            H, W = LEVELS[0]
            tiles0 = _tiles_for(0)
            if "B" in _phases:
                wp = wp0
                with tc.tile_pool(name="x0", bufs=1) as x0p, \
                     tc.tile_pool(name="st0", bufs=2) as st, \
                     tc.tile_pool(name="stg", bufs=3) as stg, \
                     tc.tile_pool(name="rfl", bufs=2, space="SBUF") as rfl, \
                     tc.tile_pool(name="hp0", bufs=2) as hp, \
                     tc.tile_pool(name="xs34", bufs=2) as xs34, \
                     tc.tile_pool(name="ys34", bufs=1) as ys34, \
                     tc.tile_pool(name="dr", bufs=2, space="DRAM") as drp, \
                     tc.tile_pool(name="ps0", bufs=2, space="PSUM") as pspool:
                    # weights first: the first conv only needs wc0 + a few
                    # input rows, so don't queue 13MB of image ahead of it
                    wc0, pc0 = load_w(wp, st, "c", 0)
                    xp = x0p.tile([128, 2, H + 2, W + 2], f32r, tag="xp0")
                    memset_borders(xp, 0)
                    # chunked load: row-range deps let layer-0 convs start
                    # as soon as their input rows land
                    for r in range(0, H, F0_REFILL_ROWS):
                        RR = min(F0_REFILL_ROWS, H - r)
                        for kh in range(2):
                            nc.gpsimd.dma_start(
                                out=xp[:, kh, r + 1 : r + RR + 1, 1 : W + 1],
                                in_=feats[0][128 * kh : 128 * (kh + 1), r : r + RR, :],
                            )

                    def f0_conv_layer(wt_t, yraw, stats_t):
                        for ti, (r0, R) in enumerate(tiles0):
                            N = R * W
                            sg = stg.tile([128, 2, 512], f32, tag="sg")
                            for mh in range(2):
                                ps = pspool.tile([128, 512], f32, tag=f"ps{mh}")
                                conv_tile(
                                    ps[:, :N].rearrange("p (r w) -> p r w", r=R),
                                    wt_t, xp, 0, ("tower", mh), r0, R,
                                )
                                nc.scalar.activation(sg[:, mh, :N], ps[:, :N], AF.Copy)
                                nc.vector.bn_stats(
                                    out=stats_t[:, mh, ti, :], in_=sg[:, mh, :N]
                                )
                            nc.sync.dma_start(
                                out=yraw[:, :, r0 * W : r0 * W + N], in_=sg[:, :, :N]
                            )

                    def f0_refill_chunk(yraw, ab, r, RR=None):
                        RR = min(RR or F0_REFILL_ROWS, H - r)
                        rt = rfl.tile([128, 2, F0_REFILL_ROWS, W], f32, tag="rt")
                        nc.sync.dma_start(
                            out=rt[:, :, :RR, :].rearrange("p a r w -> p a (r w)"),
                            in_=yraw[:, :, r * W : (r + RR) * W],
                        )
                        for h in range(2):
                            nc.scalar.activation(
                                xp[:, h, r + 1 : r + RR + 1, 1 : W + 1],
                                rt[:, h, :RR, :],
                                AF.Relu,
                                scale=ab[:, h : h + 1],
                                bias=ab[:, 2 + h : 3 + h],
                            )

                    def f0_refill(yraw, ab):
                        # small leading chunk: the next layer's first conv
                        # tile only needs ~5 rows, so publish them early
                        f0_refill_chunk(yraw, ab, 0, 4)
                        r = 4
                        while r < H:
                            f0_refill_chunk(yraw, ab, r)
                            r += F0_REFILL_ROWS

                    # levels 3/4 ride along as PE filler between f0 layers
                    xp34h = [load_34(xs34)]

                    # layer 0, both towers, off the pristine input
                    yraw_c = drp.tile([128, 2, H * W], f32, tag="yrc")
                    stats_c = st.tile([128, 2, len(tiles0), 6], f32, tag="stats0")
                    f0_conv_layer(wc0, yraw_c, stats_c)
                    wb0, pb0 = load_w(wp, st, "b", 0)
                    pb0b = st.tile([128, 2, 3], f32, tag="pkb")
                    nc.vector.tensor_copy(pb0b, pb0)
                    yraw_b = drp.tile([128, 2, H * W], f32, tag="yrb")
                    stats_b = st.tile([128, 2, len(tiles0), 6], f32, tag="stats0b")
                    f0_conv_layer(wb0, yraw_b, stats_b)
                    smalls34_layer(xs34, ys34, st, pspool, xp34h, wc0, pc0)
                    ab_b = gn_finalize(st, pspool, stats_b, pb0b, tiles0, W, "b")
                    ab_c = gn_finalize(st, pspool, stats_c, pc0, tiles0, W, "0")
                    f0_refill(yraw_c, ab_c)

                    for t in ("c", "b"):
                        lo = 1 if t == "c" else 0
                        for L in range(lo, NUM_CONVS):
                            wt_t, pk_t = load_w(wp, st, t, L)
                            if L >= 1:
                                yraw = drp.tile([128, 2, H * W], f32, tag="yrc")
                                stats_t = st.tile(
                                    [128, 2, len(tiles0), 6], f32, tag="stats0"
                                )
                                f0_conv_layer(wt_t, yraw, stats_t)
                            smalls34_layer(
                                xs34, ys34, st, pspool, xp34h, wt_t, pk_t
                            )
                            if L >= 1:
                                ab = gn_finalize(
                                    st, pspool, stats_t, pk_t, tiles0, W, "0"
                                )
                                f0_refill(yraw, ab)
                        wh = load_head_w(wp, t)
                        if t == "c":
                            # bbox path restarts from the parked layer-0 output;
                            # its refill chunks interleave with cls head tiles
                            # right after each chunk's last WAR reader
                            inter = {}
                            nt0 = len(tiles0)
                            for k, r in enumerate(range(0, H, F0_REFILL_ROWS)):
                                last_reader = min(
                                    nt0 - 1, (r + F0_REFILL_ROWS) // ROWS[0]
                                )
                                inter.setdefault(last_reader, []).append(r)

                            def mk(rs):
                                return lambda: [
                                    f0_refill_chunk(yraw_b, ab_b, r) for r in rs
                                ]

                            inter = {ti: mk(rs) for ti, rs in inter.items()}
                            head_convs(xp, 0, pspool, hp, wh, None, interleave=inter)
                            heads34(pspool, hp, xp34h, wh, None)
                            # fresh l3/l4 inputs for the bbox tower
                            xp34h = [load_34(xs34)]
                        else:
                            head_convs(xp, 0, pspool, hp, None, wh)
                            heads34(pspool, hp, xp34h, None, wh)

            wp0_pool.__exit__(None, None, None)

    nc.compile()
    return nc


# ---------------------------------------------------------------- entry
_CACHE = {}


def kernel(f0, f1, f2, f3, f4, params):
    from concourse.bass_utils import run_bass_kernel_spmd

    feats = [_np(f0), _np(f1), _np(f2), _np(f3), _np(f4)]
    scales = _np(params["scales"])

    wmap = {}
    for t, key in (("c", "cls_tower"), ("b", "bbox_tower")):
        for L, (W, b, g, be) in enumerate(params[key]):
            wmap[f"w{t}{L}"] = _prep_tower_w(W)
            pk = np.stack([_np(g), _np(be), _np(b)], axis=1)  # [256, 3]
            wmap[f"p{t}{L}"] = np.ascontiguousarray(
                pk.reshape(2, 128, 3).transpose(1, 0, 2).reshape(128, 6)
            )
    wmap["wch"], _ = _prep_head_w([params["cls_W"]])
    wmap["wbh"], _ = _prep_head_w([params["box_W"], params["ctr_W"]])
    wmap["hbc"] = _np(params["cls_b"]).reshape(16, 1)
    hbb = np.zeros((4, 5), np.float32)
    for l in range(5):
        hbb[:, l] = _np(params["box_b"]) * float(scales[l]) * float(STRIDES[l])
    wmap["hbb"] = hbb
    gavg = np.zeros((128, 128), np.float32)
    for g in range(16):
        gavg[8 * g : 8 * (g + 1), 8 * g : 8 * (g + 1)] = 0.125
    wmap["gavg"] = gavg

    key = scales.tobytes()
    if key not in _CACHE:
        _CACHE[key] = build_program(scales)
    nc = _CACHE[key]

    in_maps = []
    for b in range(NCORES):
        m = {f"f{l}": feats[l][b] for l in range(5)}
        m.update(wmap)
        in_maps.append(m)

    # the axon/PJRT execute occasionally faults transiently
    # (NRT_EXEC_UNIT_UNRECOVERABLE); retry a couple of times
    import time

    last = None
    for attempt in range(3):
        try:
            res = run_bass_kernel_spmd(nc, in_maps, core_ids=list(range(NCORES)))
            break
        except Exception as e:  # noqa: BLE001
            last = e
            if attempt == 2:
                raise
            time.sleep(5.0)
    out = np.stack([res.results[b]["out"] for b in range(NCORES)], axis=0)
    return out, _locations()
